# revision 52
# baseline (speedup 1.0000x reference)
"""Trainium2 Bass kernel for nn_DGraph_GAT (dense latent-graph GraphConv).

Strategy (8 NeuronCores, row-sharded over the 8192 nodes, 1024 nodes/core):
  - feature-major ("transposed") layouts everywhere: tensors are [feat, node]
  - per-core MLP encoder on the local 1024 nodes (layer1 float32r, 2-3 fp16)
  - AllGather of fp16 h and sq(h) (66KB/rank); X1 = h@gw1 and X2 = h1@gw2
    lhsT tiles are recomputed per core from the gathered fp16 data
  - cdist+sigmoid adjacency: an augmented K=34 fp16 matmul produces
    t^2*d2[j,i] directly in PSUM; ACT Sqrt (with +eps bias, clamps fp
    rounding negatives) evacuates PSUM -> fp16 A-buffer; one batched ACT
    Sigmoid pass turns it into A = sigmoid(t*(d+theta)) in place.
    A^T (column block, symmetric) stays fp16-resident in SBUF (16.8MB).
  - GraphConv layers: A^T block streamed through the PE as the moving
    operand against the small gathered X1/X2 matrices (PSUM-accumulated).
  - second tiny AllGather for X2 = h1@gw2, then the output MLP.
  - host side: shard/transpose inputs, gather per-core [16,1024] outputs.
"""

import os

import numpy as np

import concourse.bacc as bacc
import concourse.bass_utils as bass_utils
import concourse.mybir as mybir
import concourse.tile as tile
from concourse.tile_rust import add_dep_helper

F32 = mybir.dt.float32
F32R = mybir.dt.float32r
F16 = mybir.dt.float16
AF = mybir.ActivationFunctionType

NCORES = 8
N = 8192
LOC = N // NCORES          # 1024 nodes per core
JT = N // 128              # 64 j-tiles of 128 global nodes
IN_F, H_F, OUT_F = 512, 256, 32
NCLS = 16
INV = 1.0 / N
EPS_REL = 0.3              # sqrt clamp bias, as a fraction of t^2

# gather-1 flat layout (fp16 elements per rank)
G1_H = OUT_F * LOC         # 32768  h rows
G1_SQ = LOC                # 1024   sq row
G1_TOT = G1_H + G1_SQ      # 33792
G2_TOT = LOC * 16          # 16384  h1 rows (f n)


def _build(t: float, theta: float, sim: bool = False):
    tsq = t * t
    sgn = 1.0 if t >= 0 else -1.0
    nc = bacc.Bacc("TRN2", target_bir_lowering=False, debug=False,
                   enable_asserts=False,
                   num_devices=1 if sim else NCORES)

    def allgather(g_in, g_out, nelem):
        if sim:
            # cost-model build: stand in for the collective with local copies
            # into every rank segment (so unpack deps behave like the real AG)
            for r in range(NCORES):
                nc.sync.dma_start(
                    g_out[r * nelem:(r + 1) * nelem].rearrange(
                        "(o n) -> o n", o=1),
                    g_in[:].rearrange("(o n) -> o n", o=1))
        else:
            nc.gpsimd.collective_compute(
                "AllGather", mybir.AluOpType.bypass,
                replica_groups=[list(range(NCORES))],
                ins=[g_in.opt()], outs=[g_out.opt()])

    # ---- kernel I/O -----------------------------------------------------
    xt_d = nc.dram_tensor("xt", [IN_F, LOC], F32R, kind="ExternalInput")
    w1_d = nc.dram_tensor("w1r", [IN_F, H_F], F32R, kind="ExternalInput")
    b1_d = nc.dram_tensor("b1", [H_F, 1], F32, kind="ExternalInput")
    w2_d = nc.dram_tensor("w2h", [H_F, H_F], F16, kind="ExternalInput")
    b2_d = nc.dram_tensor("b2", [H_F, 1], F32, kind="ExternalInput")
    w3_d = nc.dram_tensor("w3h", [H_F, OUT_F], F16, kind="ExternalInput")
    b3_d = nc.dram_tensor("b3", [OUT_F, 1], F32, kind="ExternalInput")
    gw1_d = nc.dram_tensor("gw1h", [32, 16], F16, kind="ExternalInput")
    gb1_d = nc.dram_tensor("gb1", [16, 1], F32, kind="ExternalInput")
    gw2_d = nc.dram_tensor("gw2h", [16, 8], F16, kind="ExternalInput")
    gb2_d = nc.dram_tensor("gb2", [8, 1], F32, kind="ExternalInput")
    lw1_d = nc.dram_tensor("lw1h", [8, 16], F16, kind="ExternalInput")
    lb1_d = nc.dram_tensor("lb1", [16, 1], F32, kind="ExternalInput")
    lw2_d = nc.dram_tensor("lw2h", [16, 16], F16, kind="ExternalInput")
    lb2_d = nc.dram_tensor("lb2", [NCLS, 1], F32, kind="ExternalInput")
    out_d = nc.dram_tensor("outT", [NCLS, LOC], F32, kind="ExternalOutput")

    with tile.TileContext(nc) as tc:
        with (
            tc.tile_pool(name="dram", bufs=1, space="DRAM") as dram,
            tc.tile_pool(name="outer", bufs=1) as outer,
        ):
            # ---- persistent SBUF tensors -------------------------------
            X_big = outer.tile([128, JT * LOC], F16)      # A^T block / d
            hGT = outer.tile([34, N], F16)                # G lhsT (h,sq,1)
            rhs_G = outer.tile([34, LOC], F16)            # G moving operand
            X1_sb = outer.tile([128, JT * 16], F16)       # X1 lhsT tiles
            X2_sb = outer.tile([128, JT * 8], F16)        # X2 lhsT tiles
            hT3 = outer.tile([OUT_F, LOC], F32)           # local h (fp32)
            hb = outer.tile([OUT_F, LOC], F16)            # local h (fp16)
            sq_sb = outer.tile([1, LOC], F32)
            sq16 = outer.tile([1, LOC], F16)
            h1T = outer.tile([16, LOC], F16)
            h2T = outer.tile([8, LOC], F16)
            h3T = outer.tile([16, LOC], F16)
            outT = outer.tile([NCLS, LOC], F32)
            gw1_sb = outer.tile([32, 16], F16)
            gb1_sb = outer.tile([16, 1], F32)
            gw2_sb = outer.tile([16, 8], F16)
            gb2_sb = outer.tile([8, 1], F32)
            lw1_sb = outer.tile([8, 16], F16)
            lb1_sb = outer.tile([16, 1], F32)
            lw2_sb = outer.tile([16, 16], F16)
            lb2_sb = outer.tile([NCLS, 1], F32)
            eps_sb = outer.tile([128, 1], F32)
            sigb_sb = outer.tile([128, 1], F32)
            ones32 = outer.tile([32, 1], F16)
            ones_row = outer.tile([1, LOC], F16)
            tsq_row = outer.tile([1, LOC], F16)

            nc.gpsimd.memset(eps_sb[:], tsq * EPS_REL)
            nc.gpsimd.memset(sigb_sb[:], t * theta)
            nc.gpsimd.memset(ones32[:], 1.0)
            nc.gpsimd.memset(ones_row[:], 1.0)
            nc.gpsimd.memset(tsq_row[:], tsq)

            g1i = dram.tile([G1_TOT], F16)
            g1o = dram.tile([NCORES * G1_TOT], F16)
            g2i = dram.tile([G2_TOT], F16)
            g2o = dram.tile([NCORES * G2_TOT], F16)

            # ================= phase A: local MLP =======================
            with (
                tc.tile_pool(name="mlp", bufs=1) as mlp,
                tc.tile_pool(name="xts", bufs=3) as xts,
                tc.tile_pool(name="mlp_ps", bufs=4, space="PSUM") as mps,
            ):
                w1_sb = mlp.tile([128, 4 * H_F], F32R)
                w2_sb = mlp.tile([128, 2 * H_F], F16)
                w3_sb = mlp.tile([128, 2 * OUT_F], F16)
                b1_sb = mlp.tile([128, 2], F32)
                b2_sb = mlp.tile([128, 2], F32)
                b3_sb = mlp.tile([OUT_F, 1], F32)
                hT1a = mlp.tile([128, LOC], F16)
                hT1b = mlp.tile([128, LOC], F16)
                hT2a = mlp.tile([128, LOC], F16)
                hT2b = mlp.tile([128, LOC], F16)
                sqh = mlp.tile([OUT_F, LOC], F16)



                hT1 = (hT1a, hT1b)
                hT2 = (hT2a, hT2b)
                # layer 1: [512,256] @ xT, float32r; k-outer, 4 accumulators
                pm1 = [[mps.tile([128, 512], F32, tag="pm", name=f"pm1_{m}{n}")
                        for n in range(2)] for m in range(2)]
                for k in range(4):
                    nc.sync.dma_start(w1_sb[:, k * H_F:(k + 1) * H_F],
                                      w1_d[k * 128:(k + 1) * 128, :])
                    xt_k = xts.tile([128, LOC], F32R, tag="xtk")
                    nc.sync.dma_start(xt_k[:], xt_d[k * 128:(k + 1) * 128, :])
                    if k == 0:
                        for kk in range(2):
                            nc.sync.dma_start(b1_sb[:, kk:kk + 1],
                                              b1_d[kk * 128:(kk + 1) * 128, :])
                    if k == 1:
                        for kk in range(2):
                            nc.sync.dma_start(
                                w2_sb[:, kk * H_F:(kk + 1) * H_F],
                                w2_d[kk * 128:(kk + 1) * 128, :])
                            nc.sync.dma_start(b2_sb[:, kk:kk + 1],
                                              b2_d[kk * 128:(kk + 1) * 128, :])
                    if k == 2:
                        for kk in range(2):
                            nc.sync.dma_start(
                                w3_sb[:, kk * OUT_F:(kk + 1) * OUT_F],
                                w3_d[kk * 128:(kk + 1) * 128, :])
                        nc.sync.dma_start(b3_sb[:], b3_d[:])
                    for m in range(2):
                        for n in range(2):
                            nc.tensor.matmul(
                                pm1[m][n][:],
                                w1_sb[:, k * H_F + m * 128:k * H_F + (m + 1) * 128],
                                xt_k[:, n * 512:(n + 1) * 512],
                                start=(k == 0), stop=(k == 3))
                for m in range(2):
                    for n in range(2):
                        nc.scalar.activation(
                            hT1[m][:, n * 512:(n + 1) * 512], pm1[m][n][:],
                            AF.Relu, bias=b1_sb[:, m:m + 1], scale=1.0)
                # layer 2: fp16
                for m in range(2):
                    for n in range(2):
                        pm = mps.tile([128, 512], F32, tag="pm")
                        for k in range(2):
                            nc.tensor.matmul(
                                pm[:],
                                w2_sb[:, k * H_F + m * 128:k * H_F + (m + 1) * 128],
                                hT1[k][:, n * 512:(n + 1) * 512],
                                start=(k == 0), stop=(k == 1))
                        nc.scalar.activation(
                            hT2[m][:, n * 512:(n + 1) * 512], pm[:], AF.Relu,
                            bias=b2_sb[:, m:m + 1], scale=1.0)
                # layer 3: fp16 -> hT3 [32, 1024] fp32 (no relu)
                for n in range(2):
                    pm = mps.tile([OUT_F, 512], F32, tag="pm")
                    for k in range(2):
                        nc.tensor.matmul(
                            pm[:],
                            w3_sb[:, k * OUT_F:(k + 1) * OUT_F],
                            hT2[k][:, n * 512:(n + 1) * 512],
                            start=(k == 0), stop=(k == 1))
                    nc.scalar.activation(
                        hT3[:, n * 512:(n + 1) * 512], pm[:], AF.Identity,
                        bias=b3_sb[:], scale=1.0)

                # fp16 copy of h + sq = sum_f h16^2 (fp16 matmul w/ ones)
                nc.vector.tensor_copy(hb[:], hT3[:])
                nc.vector.tensor_mul(sqh[:], hb[:], hb[:])
                for n in range(2):
                    pm = mps.tile([1, 512], F32, tag="pm")
                    nc.tensor.matmul(pm[:], ones32[:],
                                     sqh[:, n * 512:(n + 1) * 512],
                                     start=True, stop=True)
                    nc.vector.tensor_copy(sq_sb[:, n * 512:(n + 1) * 512], pm[:])
                nc.vector.tensor_copy(sq16[:], sq_sb[:])

            # ================= gather 1 =================================
            nc.sync.dma_start(
                g1i[0:G1_H].rearrange("(f n) -> f n", f=OUT_F), hb[:])
            nc.sync.dma_start(
                g1i[G1_H:G1_H + G1_SQ].rearrange("(o n) -> o n", o=1), sq16[:])

            # moving operand rows: 0-31 = -2 t^2 h_loc (pairs h_glob),
            # 32 = t^2 sq_loc (pairs ones), 33 = t^2 (pairs sq_glob)
            # => psum = t^2 * d2[j, i]   (local data only — build pre-gather)
            nc.vector.tensor_scalar_mul(rhs_G[0:32, :], hT3[:], -2.0 * tsq)
            nc.vector.tensor_scalar_mul(rhs_G[32:33, :], sq_sb[:], tsq)

            # static rows + tail weights: fill the collective-latency window
            nc.sync.dma_start(rhs_G[33:34, :], tsq_row[:])
            for r in range(NCORES):
                nc.sync.dma_start(hGT[32:33, r * LOC:(r + 1) * LOC],
                                  ones_row[:])
            nc.gpsimd.dma_start(gw1_sb[:], gw1_d[:])
            nc.gpsimd.dma_start(gb1_sb[:], gb1_d[:])
            nc.gpsimd.dma_start(gw2_sb[:], gw2_d[:])
            nc.gpsimd.dma_start(gb2_sb[:], gb2_d[:])
            nc.gpsimd.dma_start(lw1_sb[:], lw1_d[:])
            nc.gpsimd.dma_start(lb1_sb[:], lb1_d[:])
            nc.gpsimd.dma_start(lw2_sb[:], lw2_d[:])
            nc.gpsimd.dma_start(lb2_sb[:], lb2_d[:])

            allgather(g1i, g1o, G1_TOT)
            g1o_v = g1o[:].rearrange("(r q) -> r q", r=NCORES)
            nc.sync.dma_start(
                hGT[33:34, :].rearrange("o (r n) -> o r n", r=NCORES),
                g1o_v[:, G1_H:G1_H + G1_SQ].rearrange("r (o n) -> o r n", o=1))
            for r in range(NCORES):
                nc.sync.dma_start(
                    hGT[0:OUT_F, r * LOC:(r + 1) * LOC],
                    g1o_v[r, 0:G1_H].rearrange("(f n) -> f n", f=OUT_F))

            # ================= phase B: adjacency =======================
            sqrt_insts = []
            with tc.tile_pool(name="g_ps", bufs=2, space="PSUM") as gps:
                # two j-tiles per PSUM tile (4 banks x 2 bufs = all of PSUM)
                for jj in range(JT // 2):
                    pg = gps.tile([128, 2 * LOC], F32, tag="pg")
                    for half in range(4):
                        nc.tensor.matmul(
                            pg[:, half * 512:(half + 1) * 512],
                            hGT[:, 2 * jj * 128 + (half // 2) * 128:
                                2 * jj * 128 + (half // 2) * 128 + 128],
                            rhs_G[:, (half % 2) * 512:(half % 2) * 512 + 512],
                            start=True, stop=True, skip_group_check=True)
                    inst = nc.scalar.activation(
                        X_big[:, 2 * jj * LOC:(2 * jj + 2) * LOC], pg[:],
                        AF.Sqrt, bias=eps_sb[:], scale=1.0)
                    sqrt_insts.append(inst)

            with (
                tc.tile_pool(name="acc_ps", bufs=2, space="PSUM") as aps,
                tc.tile_pool(name="x2_ps", bufs=2, space="PSUM") as xps,
                tc.tile_pool(name="x1_ps", bufs=2, space="PSUM") as x1ps,
                tc.tile_pool(name="xts2", bufs=2) as xts2,
            ):
                # X1 lhsT tiles from gathered fp16 h — PE is idle during the
                # sigmoid window, so schedule these after the G matmuls
                for g in range(8):
                    px1 = x1ps.tile([128, 128], F32, tag="px1", name=f"px1_{g}")
                    for l in range(8):
                        j = g * 8 + l
                        nc.tensor.matmul(px1[:, l * 16:(l + 1) * 16],
                                         hGT[0:32, j * 128:(j + 1) * 128],
                                         gw1_sb[:], start=True, stop=True,
                                         skip_group_check=True)
                    nc.vector.tensor_copy(
                        X1_sb[:, g * 128:(g + 1) * 128], px1[:])

                # batched sigmoid pass (after ALL sqrts: one table switch)
                for c in range(16):
                    ap = X_big[:, c * 4096:(c + 1) * 4096]
                    inst = nc.scalar.activation(ap, ap, AF.Sigmoid,
                                                bias=sigb_sb[:], scale=sgn)
                    add_dep_helper(inst.ins, sqrt_insts[-1].ins, sync=False,
                                   reason="batch ACT table sets")

                # ---- GraphConv layer 1:  (A^T @ X1) * inv + gb1 --------
                o1a = aps.tile([16, 512], F32, tag="acc")
                o1b = aps.tile([16, 512], F32, tag="acc")
                o1 = (o1a, o1b)
                for j in range(JT):
                    for h in range(2):
                        nc.tensor.matmul(
                            o1[h][:], X1_sb[:, j * 16:(j + 1) * 16],
                            X_big[:, j * LOC + h * 512:j * LOC + h * 512 + 512],
                            start=(j == 0), stop=(j == JT - 1))
                for h in range(2):
                    nc.scalar.activation(
                        h1T[:, h * 512:(h + 1) * 512], o1[h][:], AF.Relu,
                        bias=gb1_sb[:], scale=INV)

                # ---- gather 2: ship h1 (fp16), then X2 = h1 @ gw2 ------
                nc.sync.dma_start(
                    g2i[:].rearrange("(f n) -> f n", f=16), h1T[:])
                allgather(g2i, g2o, G2_TOT)
                for r in range(NCORES):
                    h1r = xts2.tile([16, LOC], F16, tag="h1r", name=f"h1r_{r}")
                    nc.sync.dma_start(
                        h1r[:],
                        g2o[r * G2_TOT:(r + 1) * G2_TOT].rearrange(
                            "(f n) -> f n", f=16))
                    px = xps.tile([128, 64], F32, tag="px2", name=f"px2_{r}")
                    for l in range(8):
                        nc.tensor.matmul(px[:, l * 8:(l + 1) * 8],
                                         h1r[:, l * 128:(l + 1) * 128],
                                         gw2_sb[:], start=True, stop=True,
                                         skip_group_check=True)
                    nc.vector.tensor_copy(X2_sb[:, r * 64:(r + 1) * 64], px[:])

                # ---- GraphConv layer 2 ---------------------------------
                o2a = aps.tile([8, 512], F32, tag="acc")
                o2b = aps.tile([8, 512], F32, tag="acc")
                o2 = (o2a, o2b)
                for j in range(JT):
                    for h in range(2):
                        nc.tensor.matmul(
                            o2[h][:], X2_sb[:, j * 8:(j + 1) * 8],
                            X_big[:, j * LOC + h * 512:j * LOC + h * 512 + 512],
                            start=(j == 0), stop=(j == JT - 1))
                for h in range(2):
                    nc.scalar.activation(
                        h2T[:, h * 512:(h + 1) * 512], o2[h][:], AF.Relu,
                        bias=gb2_sb[:], scale=INV)

                # ---- output MLP ----------------------------------------
                for n in range(2):
                    ph = aps.tile([16, 512], F32, tag="acc")
                    nc.tensor.matmul(ph[:], lw1_sb[:],
                                     h2T[:, n * 512:(n + 1) * 512],
                                     start=True, stop=True)
                    nc.scalar.activation(
                        h3T[:, n * 512:(n + 1) * 512], ph[:], AF.Relu,
                        bias=lb1_sb[:], scale=1.0)
                for n in range(2):
                    po = aps.tile([NCLS, 512], F32, tag="acc")
                    nc.tensor.matmul(po[:], lw2_sb[:],
                                     h3T[:, n * 512:(n + 1) * 512],
                                     start=True, stop=True)
                    nc.scalar.activation(
                        outT[:, n * 512:(n + 1) * 512], po[:], AF.Identity,
                        bias=lb2_sb[:], scale=1.0)
                nc.sync.dma_start(out_d[:], outT[:])

    nc.compile()
    return nc


_CACHE = {}


def _get_nc(t: float, theta: float):
    key = (t, theta)
    if key not in _CACHE:
        _CACHE[key] = _build(t, theta)
    return _CACHE[key]


def _prep_inputs(inputs):
    I = {k: np.asarray(v) for k, v in inputs.items()}
    t = float(I["t"][0, 0])
    theta = float(I["theta"][0, 0])
    xt = np.ascontiguousarray(I["x"].astype(np.float32).T)   # [512, 8192]
    shared = {
        "w1r": np.ascontiguousarray(I["w1"].astype(np.float32)),
        "b1": I["b1"].astype(np.float32).reshape(-1, 1),
        "w2h": np.ascontiguousarray(I["w2"].astype(np.float16)),
        "b2": I["b2"].astype(np.float32).reshape(-1, 1),
        "w3h": np.ascontiguousarray(I["w3"].astype(np.float16)),
        "b3": I["b3"].astype(np.float32).reshape(-1, 1),
        "gw1h": np.ascontiguousarray(I["gw1"].astype(np.float16)),
        "gb1": I["gb1"].astype(np.float32).reshape(-1, 1),
        "gw2h": np.ascontiguousarray(I["gw2"].astype(np.float16)),
        "gb2": I["gb2"].astype(np.float32).reshape(-1, 1),
        "lw1h": np.ascontiguousarray(I["lw1"].astype(np.float16)),
        "lb1": I["lb1"].astype(np.float32).reshape(-1, 1),
        "lw2h": np.ascontiguousarray(I["lw2"].astype(np.float16)),
        "lb2": I["lb2"].astype(np.float32).reshape(-1, 1),
    }
    in_maps = []
    for c in range(NCORES):
        m = dict(shared)
        m["xt"] = np.ascontiguousarray(xt[:, c * LOC:(c + 1) * LOC])
        in_maps.append(m)
    return t, theta, in_maps


def _execute(inputs, **run_kwargs):
    t, theta, in_maps = _prep_inputs(inputs)
    nc = _get_nc(t, theta)
    try:
        res = bass_utils.run_bass_kernel_spmd(
            nc, in_maps, core_ids=list(range(NCORES)), **run_kwargs)
    except ModuleNotFoundError:
        # NTFF trace hook unavailable in this container — run untraced
        os.environ["BASS_NEVER_TRACE"] = "1"
        run_kwargs.pop("trace", None)
        res = bass_utils.run_bass_kernel_spmd(
            nc, in_maps, core_ids=list(range(NCORES)), **run_kwargs)
    out = np.concatenate(
        [res.results[c]["outT"].T for c in range(NCORES)], axis=0)
    return np.ascontiguousarray(out.astype(np.float32)), res


def kernel(**inputs) -> np.ndarray:
    out, _ = _execute(inputs)
    return out


# revision 60
# speedup vs baseline: 1.0484x; 1.0484x over previous
"""Trainium2 Bass kernel for nn_DGraph_GAT (dense latent-graph GraphConv).

Strategy (8 NeuronCores, row-sharded over the 8192 nodes, 1024 nodes/core):
  - feature-major ("transposed") layouts everywhere: tensors are [feat, node]
  - per-core MLP encoder on the local 1024 nodes (layer1 float32r, 2-3 fp16)
  - AllGather of fp16 h and sq(h) (66KB/rank); X1 = h@gw1 and X2 = h1@gw2
    lhsT tiles are recomputed per core from the gathered fp16 data
  - cdist+sigmoid adjacency: an augmented K=34 fp16 matmul produces
    t^2*d2[j,i] directly in PSUM; ACT Sqrt (with +eps bias, clamps fp
    rounding negatives) evacuates PSUM -> fp16 A-buffer; one batched ACT
    Sigmoid pass turns it into A = sigmoid(t*(d+theta)) in place.
    A^T (column block, symmetric) stays fp16-resident in SBUF (16.8MB).
  - GraphConv layers: A^T block streamed through the PE as the moving
    operand against the small gathered X1/X2 matrices (PSUM-accumulated).
  - second tiny AllGather for X2 = h1@gw2, then the output MLP.
  - host side: shard/transpose inputs, gather per-core [16,1024] outputs.
"""

import os

import numpy as np

import concourse.bacc as bacc
import concourse.bass_utils as bass_utils
import concourse.mybir as mybir
import concourse.tile as tile
from concourse.tile_rust import add_dep_helper

F32 = mybir.dt.float32
F32R = mybir.dt.float32r
F16 = mybir.dt.float16
AF = mybir.ActivationFunctionType

NCORES = 8
N = 8192
LOC = N // NCORES          # 1024 nodes per core
JT = N // 128              # 64 j-tiles of 128 global nodes
IN_F, H_F, OUT_F = 512, 256, 32
NCLS = 16
INV = 1.0 / N
EPS_REL = 0.3              # sqrt clamp bias, as a fraction of t^2

# gather-1 flat layout (fp16 elements per rank)
G1_H = OUT_F * LOC         # 32768  h rows
G1_SQ = LOC                # 1024   sq row
G1_TOT = G1_H + G1_SQ      # 33792
G2_TOT = LOC * 16          # 16384  h1 rows (f n)


def _build(t: float, theta: float, sim: bool = False):
    tsq = t * t
    sgn = 1.0 if t >= 0 else -1.0
    nc = bacc.Bacc("TRN2", target_bir_lowering=False, debug=False,
                   enable_asserts=False,
                   num_devices=1 if sim else NCORES)

    def allgather(g_in, g_out, nelem):
        if sim:
            # cost-model build: stand in for the collective with local copies
            # into every rank segment (so unpack deps behave like the real AG)
            for r in range(NCORES):
                nc.sync.dma_start(
                    g_out[r * nelem:(r + 1) * nelem].rearrange(
                        "(o n) -> o n", o=1),
                    g_in[:].rearrange("(o n) -> o n", o=1))
        else:
            nc.gpsimd.collective_compute(
                "AllGather", mybir.AluOpType.bypass,
                replica_groups=[list(range(NCORES))],
                ins=[g_in.opt()], outs=[g_out.opt()])

    # ---- kernel I/O -----------------------------------------------------
    xt_d = nc.dram_tensor("xt", [IN_F, LOC], F32R, kind="ExternalInput")
    w1_d = nc.dram_tensor("w1r", [IN_F, H_F], F32R, kind="ExternalInput")
    b1_d = nc.dram_tensor("b1", [H_F, 1], F32, kind="ExternalInput")
    w2_d = nc.dram_tensor("w2h", [H_F, H_F], F16, kind="ExternalInput")
    b2_d = nc.dram_tensor("b2", [H_F, 1], F32, kind="ExternalInput")
    w3_d = nc.dram_tensor("w3h", [H_F, OUT_F], F16, kind="ExternalInput")
    b3_d = nc.dram_tensor("b3", [OUT_F, 1], F32, kind="ExternalInput")
    gw1_d = nc.dram_tensor("gw1h", [32, 16], F16, kind="ExternalInput")
    gb1_d = nc.dram_tensor("gb1", [16, 1], F32, kind="ExternalInput")
    gw2_d = nc.dram_tensor("gw2h", [16, 8], F16, kind="ExternalInput")
    gb2_d = nc.dram_tensor("gb2", [8, 1], F32, kind="ExternalInput")
    lw1_d = nc.dram_tensor("lw1h", [8, 16], F16, kind="ExternalInput")
    lb1_d = nc.dram_tensor("lb1", [16, 1], F32, kind="ExternalInput")
    lw2_d = nc.dram_tensor("lw2h", [16, 16], F16, kind="ExternalInput")
    lb2_d = nc.dram_tensor("lb2", [NCLS, 1], F32, kind="ExternalInput")
    out_d = nc.dram_tensor("outT", [NCLS, LOC], F32, kind="ExternalOutput")

    with tile.TileContext(nc) as tc:
        with (
            tc.tile_pool(name="dram", bufs=1, space="DRAM") as dram,
            tc.tile_pool(name="outer", bufs=1) as outer,
        ):
            # ---- persistent SBUF tensors -------------------------------
            X_big = outer.tile([128, JT * LOC], F16)      # A^T block / d
            hGT = outer.tile([34, N], F16)                # G lhsT (h,sq,1)
            rhs_G = outer.tile([34, LOC], F16)            # G moving operand
            X1_sb = outer.tile([128, JT * 16], F16)       # X1 lhsT tiles
            X2_sb = outer.tile([128, JT * 8], F16)        # X2 lhsT tiles
            hT3 = outer.tile([OUT_F, LOC], F32)           # local h (fp32)
            hb = outer.tile([OUT_F, LOC], F16)            # local h (fp16)
            sq_sb = outer.tile([1, LOC], F32)
            sq16 = outer.tile([1, LOC], F16)
            h1T = outer.tile([16, LOC], F16)
            h2T = outer.tile([8, LOC], F16)
            h3T = outer.tile([16, LOC], F16)
            outT = outer.tile([NCLS, LOC], F32)
            gw1_sb = outer.tile([32, 16], F16)
            gb1_sb = outer.tile([16, 1], F32)
            gw2_sb = outer.tile([16, 8], F16)
            gb2_sb = outer.tile([8, 1], F32)
            lw1_sb = outer.tile([8, 16], F16)
            lb1_sb = outer.tile([16, 1], F32)
            lw2_sb = outer.tile([16, 16], F16)
            lb2_sb = outer.tile([NCLS, 1], F32)
            eps_sb = outer.tile([128, 1], F32)
            sigb_sb = outer.tile([128, 1], F32)
            ones32 = outer.tile([32, 1], F16)
            ones_row = outer.tile([1, LOC], F16)
            tsq_row = outer.tile([1, LOC], F16)

            nc.gpsimd.memset(eps_sb[:], tsq * EPS_REL)
            nc.gpsimd.memset(sigb_sb[:], t * theta)
            nc.gpsimd.memset(ones32[:], 1.0)
            nc.gpsimd.memset(ones_row[:], 1.0)
            nc.gpsimd.memset(tsq_row[:], tsq)

            g1i = dram.tile([G1_TOT], F16)
            g1o = dram.tile([NCORES * G1_TOT], F16)
            g2ai = dram.tile([G2_TOT // 2], F16)
            g2ao = dram.tile([NCORES * G2_TOT // 2], F16)
            g2bi = dram.tile([G2_TOT // 2], F16)
            g2bo = dram.tile([NCORES * G2_TOT // 2], F16)

            # ================= phase A: local MLP =======================
            with (
                tc.tile_pool(name="mlp", bufs=1) as mlp,
                tc.tile_pool(name="xts", bufs=3) as xts,
                tc.tile_pool(name="mlp_ps", bufs=4, space="PSUM") as mps,
            ):
                w1_sb = mlp.tile([128, 4 * H_F], F32R)
                w2_sb = mlp.tile([128, 2 * H_F], F16)
                w3_sb = mlp.tile([128, 2 * OUT_F], F16)
                b1_sb = mlp.tile([128, 2], F32)
                b2_sb = mlp.tile([128, 2], F32)
                b3_sb = mlp.tile([OUT_F, 1], F32)
                hT1a = mlp.tile([128, LOC], F16)
                hT1b = mlp.tile([128, LOC], F16)
                hT2a = mlp.tile([128, LOC], F16)
                hT2b = mlp.tile([128, LOC], F16)
                sqh = mlp.tile([OUT_F, LOC], F16)



                hT1 = (hT1a, hT1b)
                hT2 = (hT2a, hT2b)
                # layer 1: [512,256] @ xT, float32r; k-outer, 4 accumulators
                pm1 = [[mps.tile([128, 512], F32, tag="pm", name=f"pm1_{m}{n}")
                        for n in range(2)] for m in range(2)]
                for k in range(4):
                    nc.sync.dma_start(w1_sb[:, k * H_F:(k + 1) * H_F],
                                      w1_d[k * 128:(k + 1) * 128, :])
                    xt_k = xts.tile([128, LOC], F32R, tag="xtk")
                    nc.sync.dma_start(xt_k[:], xt_d[k * 128:(k + 1) * 128, :])
                    if k == 0:
                        for kk in range(2):
                            nc.sync.dma_start(b1_sb[:, kk:kk + 1],
                                              b1_d[kk * 128:(kk + 1) * 128, :])
                    if k == 1:
                        for kk in range(2):
                            nc.sync.dma_start(
                                w2_sb[:, kk * H_F:(kk + 1) * H_F],
                                w2_d[kk * 128:(kk + 1) * 128, :])
                            nc.sync.dma_start(b2_sb[:, kk:kk + 1],
                                              b2_d[kk * 128:(kk + 1) * 128, :])
                    if k == 2:
                        for kk in range(2):
                            nc.sync.dma_start(
                                w3_sb[:, kk * OUT_F:(kk + 1) * OUT_F],
                                w3_d[kk * 128:(kk + 1) * 128, :])
                        nc.sync.dma_start(b3_sb[:], b3_d[:])
                    for m in range(2):
                        for n in range(2):
                            nc.tensor.matmul(
                                pm1[m][n][:],
                                w1_sb[:, k * H_F + m * 128:k * H_F + (m + 1) * 128],
                                xt_k[:, n * 512:(n + 1) * 512],
                                start=(k == 0), stop=(k == 3))
                for m in range(2):
                    for n in range(2):
                        nc.scalar.activation(
                            hT1[m][:, n * 512:(n + 1) * 512], pm1[m][n][:],
                            AF.Relu, bias=b1_sb[:, m:m + 1], scale=1.0)
                # layer 2: fp16
                for m in range(2):
                    for n in range(2):
                        pm = mps.tile([128, 512], F32, tag="pm")
                        for k in range(2):
                            nc.tensor.matmul(
                                pm[:],
                                w2_sb[:, k * H_F + m * 128:k * H_F + (m + 1) * 128],
                                hT1[k][:, n * 512:(n + 1) * 512],
                                start=(k == 0), stop=(k == 1))
                        nc.scalar.activation(
                            hT2[m][:, n * 512:(n + 1) * 512], pm[:], AF.Relu,
                            bias=b2_sb[:, m:m + 1], scale=1.0)
                # layer 3: fp16 -> hT3 [32, 1024] fp32 (no relu)
                for n in range(2):
                    pm = mps.tile([OUT_F, 512], F32, tag="pm")
                    for k in range(2):
                        nc.tensor.matmul(
                            pm[:],
                            w3_sb[:, k * OUT_F:(k + 1) * OUT_F],
                            hT2[k][:, n * 512:(n + 1) * 512],
                            start=(k == 0), stop=(k == 1))
                    nc.scalar.activation(
                        hT3[:, n * 512:(n + 1) * 512], pm[:], AF.Identity,
                        bias=b3_sb[:], scale=1.0)

                # fp16 copy of h + sq = sum_f h16^2 (fp16 matmul w/ ones)
                nc.vector.tensor_copy(hb[:], hT3[:])
                nc.vector.tensor_mul(sqh[:], hb[:], hb[:])
                for n in range(2):
                    pm = mps.tile([1, 512], F32, tag="pm")
                    nc.tensor.matmul(pm[:], ones32[:],
                                     sqh[:, n * 512:(n + 1) * 512],
                                     start=True, stop=True)
                    nc.vector.tensor_copy(sq_sb[:, n * 512:(n + 1) * 512], pm[:])
                nc.vector.tensor_copy(sq16[:], sq_sb[:])

            # ================= gather 1 =================================
            nc.sync.dma_start(
                g1i[0:G1_H].rearrange("(f n) -> f n", f=OUT_F), hb[:])
            nc.sync.dma_start(
                g1i[G1_H:G1_H + G1_SQ].rearrange("(o n) -> o n", o=1), sq16[:])

            # moving operand rows: 0-31 = -2 t^2 h_loc (pairs h_glob),
            # 32 = t^2 sq_loc (pairs ones), 33 = t^2 (pairs sq_glob)
            # => psum = t^2 * d2[j, i]   (local data only — build pre-gather)
            nc.vector.tensor_scalar_mul(rhs_G[0:32, :], hT3[:], -2.0 * tsq)
            nc.vector.tensor_scalar_mul(rhs_G[32:33, :], sq_sb[:], tsq)

            # static rows + tail weights: fill the collective-latency window
            nc.sync.dma_start(rhs_G[33:34, :], tsq_row[:])
            for r in range(NCORES):
                nc.sync.dma_start(hGT[32:33, r * LOC:(r + 1) * LOC],
                                  ones_row[:])
            nc.gpsimd.dma_start(gw1_sb[:], gw1_d[:])
            nc.gpsimd.dma_start(gb1_sb[:], gb1_d[:])
            nc.gpsimd.dma_start(gw2_sb[:], gw2_d[:])
            nc.gpsimd.dma_start(gb2_sb[:], gb2_d[:])
            nc.gpsimd.dma_start(lw1_sb[:], lw1_d[:])
            nc.gpsimd.dma_start(lb1_sb[:], lb1_d[:])
            nc.gpsimd.dma_start(lw2_sb[:], lw2_d[:])
            nc.gpsimd.dma_start(lb2_sb[:], lb2_d[:])

            allgather(g1i, g1o, G1_TOT)
            g1o_v = g1o[:].rearrange("(r q) -> r q", r=NCORES)
            nc.sync.dma_start(
                hGT[33:34, :].rearrange("o (r n) -> o r n", r=NCORES),
                g1o_v[:, G1_H:G1_H + G1_SQ].rearrange("r (o n) -> o r n", o=1))
            for r in range(NCORES):
                nc.sync.dma_start(
                    hGT[0:OUT_F, r * LOC:(r + 1) * LOC],
                    g1o_v[r, 0:G1_H].rearrange("(f n) -> f n", f=OUT_F))

            # ================= phase B: adjacency =======================
            sqrt_insts = []
            with tc.tile_pool(name="g_ps", bufs=2, space="PSUM") as gps:
                # two j-tiles per PSUM tile (4 banks x 2 bufs = all of PSUM)
                for jj in range(JT // 2):
                    pg = gps.tile([128, 2 * LOC], F32, tag="pg")
                    for half in range(4):
                        nc.tensor.matmul(
                            pg[:, half * 512:(half + 1) * 512],
                            hGT[:, 2 * jj * 128 + (half // 2) * 128:
                                2 * jj * 128 + (half // 2) * 128 + 128],
                            rhs_G[:, (half % 2) * 512:(half % 2) * 512 + 512],
                            start=True, stop=True, skip_group_check=True)
                    inst = nc.scalar.activation(
                        X_big[:, 2 * jj * LOC:(2 * jj + 2) * LOC], pg[:],
                        AF.Sqrt, bias=eps_sb[:], scale=1.0)
                    sqrt_insts.append(inst)

            with (
                tc.tile_pool(name="acc_ps", bufs=2, space="PSUM") as aps,
                tc.tile_pool(name="x2_ps", bufs=2, space="PSUM") as xps,
                tc.tile_pool(name="x1_ps", bufs=2, space="PSUM") as x1ps,
                tc.tile_pool(name="o2_ps", bufs=2, space="PSUM") as o2ps,
                tc.tile_pool(name="xts2", bufs=4) as xts2,
            ):
                # X1 lhsT tiles from gathered fp16 h — PE is idle during the
                # sigmoid window, so schedule these after the G matmuls
                for g in range(8):
                    px1 = x1ps.tile([128, 128], F32, tag="px1", name=f"px1_{g}")
                    for l in range(8):
                        j = g * 8 + l
                        nc.tensor.matmul(px1[:, l * 16:(l + 1) * 16],
                                         hGT[0:32, j * 128:(j + 1) * 128],
                                         gw1_sb[:], start=True, stop=True,
                                         skip_group_check=True)
                    nc.vector.tensor_copy(
                        X1_sb[:, g * 128:(g + 1) * 128], px1[:])

                # sigmoid + GraphConv1 + gather2, pipelined by node-half:
                # sweep half h of every A-tile, finish o1[h], relu, and fire
                # that half's h1 AllGather while the other half still runs.
                o1a = aps.tile([16, 512], F32, tag="acc")
                o1b = aps.tile([16, 512], F32, tag="acc")
                o1 = (o1a, o1b)
                o2a = o2ps.tile([8, 512], F32, tag="o2")
                o2b = o2ps.tile([8, 512], F32, tag="o2")
                o2 = (o2a, o2b)
                g2io = ((g2ai, g2ao), (g2bi, g2bo))

                def spmm2_tiles(tiles, start_j=None, stop_j=None):
                    insts = []
                    for j in tiles:
                        for ih in range(2):
                            insts.append(nc.tensor.matmul(
                                o2[ih][:], X2_sb[:, j * 8:(j + 1) * 8],
                                X_big[:, j * LOC + ih * 512:
                                      j * LOC + ih * 512 + 512],
                                start=(j == start_j), stop=(j == stop_j)))
                    return insts

                # group-a j-tiles (first 4 of each rank block), chunk-ordered
                ga_tiles = [8 * g + l for g in range(8)
                            for l in range(8) if (8 * g + l) % 8 < 4]

                def sig_spmm1_gather(h):
                    last_i2 = [None]
                    for g in range(8):
                        ap4 = X_big[:, g * 8192:(g + 1) * 8192].rearrange(
                            "p (l s n) -> p l s n", l=8, s=2)[
                            :, :, h:h + 1, :]
                        inst = nc.scalar.activation(ap4, ap4, AF.Sigmoid,
                                                    bias=sigb_sb[:], scale=sgn)
                        add_dep_helper(inst.ins, sqrt_insts[-1].ins,
                                       sync=False,
                                       reason="batch ACT table sets")
                        prev = None
                        for j in range(8 * g, 8 * g + 8):
                            prev = nc.tensor.matmul(
                                o1[h][:], X1_sb[:, j * 16:(j + 1) * 16],
                                X_big[:, j * LOC + h * 512:
                                      j * LOC + h * 512 + 512],
                                start=(j == 0), stop=(j == JT - 1))
                            if last_i2[0] is not None:
                                add_dep_helper(prev.ins, last_i2[0].ins,
                                               sync=False,
                                               reason="pin interleave order")
                                last_i2[0] = None
                        if h == 1 and g >= 3:
                            # fill PE idle slots of the sigmoid-b sweep with
                            # group-a SpMM2 tiles (X2a landed ~3 chunks ago);
                            # pin the alternation (scheduler would sink these)
                            gg = g - 3
                            ii = spmm2_tiles(ga_tiles[4 * gg:4 * gg + 4],
                                             start_j=ga_tiles[0])
                            add_dep_helper(ii[0].ins, prev.ins, sync=False,
                                           reason="pin spmm2a interleave")
                            last_i2[0] = ii[-1]
                    nc.scalar.activation(
                        h1T[:, h * 512:(h + 1) * 512], o1[h][:], AF.Relu,
                        bias=gb1_sb[:], scale=INV)
                    nc.sync.dma_start(
                        g2io[h][0][:].rearrange("(f n) -> f n", f=16),
                        h1T[:, h * 512:(h + 1) * 512])
                    allgather(g2io[h][0], g2io[h][1], G2_TOT // 2)

                def x2_prep(h):
                    for r in range(NCORES):
                        h1r = xts2.tile([16, 512], F16, tag="h1r",
                                        name=f"h1r_{h}_{r}")
                        nc.sync.dma_start(
                            h1r[:],
                            g2io[h][1][r * (G2_TOT // 2):
                                       (r + 1) * (G2_TOT // 2)].rearrange(
                                "(f n) -> f n", f=16))
                        px = xps.tile([128, 32], F32, tag="px2",
                                      name=f"px2_{h}_{r}")
                        for ll in range(4):
                            nc.tensor.matmul(px[:, ll * 8:(ll + 1) * 8],
                                             h1r[:, ll * 128:(ll + 1) * 128],
                                             gw2_sb[:], start=True, stop=True,
                                             skip_group_check=True)
                        nc.vector.tensor_copy(
                            X2_sb[:, r * 64 + h * 32:r * 64 + h * 32 + 32],
                            px[:])

                sig_spmm1_gather(0)
                x2_prep(0)              # half-a X2 ready during half-b sweep
                sig_spmm1_gather(1)     # interleaves 20 of 32 group-a tiles
                spmm2_tiles(ga_tiles[20:])      # remaining group-a tiles
                # half-b: unpack + X2 tiles + SpMM2 per rank, pipelined
                gb_last = NCORES * 8 - 1 - 0    # j = 63 is in group-b
                for r in range(NCORES):
                    h1r = xts2.tile([16, 512], F16, tag="h1r",
                                    name=f"h1rb_{r}")
                    nc.sync.dma_start(
                        h1r[:],
                        g2bo[r * (G2_TOT // 2):
                             (r + 1) * (G2_TOT // 2)].rearrange(
                            "(f n) -> f n", f=16))
                    px = xps.tile([128, 32], F32, tag="px2", name=f"px2b_{r}")
                    for ll in range(4):
                        nc.tensor.matmul(px[:, ll * 8:(ll + 1) * 8],
                                         h1r[:, ll * 128:(ll + 1) * 128],
                                         gw2_sb[:], start=True, stop=True,
                                         skip_group_check=True)
                    nc.vector.tensor_copy(
                        X2_sb[:, r * 64 + 32:r * 64 + 64], px[:])
                    spmm2_tiles([r * 8 + 4 + ll for ll in range(4)],
                                stop_j=gb_last)
                for ih in range(2):
                    nc.scalar.activation(
                        h2T[:, ih * 512:(ih + 1) * 512], o2[ih][:], AF.Relu,
                        bias=gb2_sb[:], scale=INV)

                # ---- output MLP ----------------------------------------
                for n in range(2):
                    ph = aps.tile([16, 512], F32, tag="acc")
                    nc.tensor.matmul(ph[:], lw1_sb[:],
                                     h2T[:, n * 512:(n + 1) * 512],
                                     start=True, stop=True)
                    nc.scalar.activation(
                        h3T[:, n * 512:(n + 1) * 512], ph[:], AF.Relu,
                        bias=lb1_sb[:], scale=1.0)
                for n in range(2):
                    po = aps.tile([NCLS, 512], F32, tag="acc")
                    nc.tensor.matmul(po[:], lw2_sb[:],
                                     h3T[:, n * 512:(n + 1) * 512],
                                     start=True, stop=True)
                    nc.scalar.activation(
                        outT[:, n * 512:(n + 1) * 512], po[:], AF.Identity,
                        bias=lb2_sb[:], scale=1.0)
                nc.sync.dma_start(out_d[:], outT[:])

    nc.compile()
    return nc


_CACHE = {}


def _get_nc(t: float, theta: float):
    key = (t, theta)
    if key not in _CACHE:
        _CACHE[key] = _build(t, theta)
    return _CACHE[key]


def _prep_inputs(inputs):
    I = {k: np.asarray(v) for k, v in inputs.items()}
    t = float(I["t"][0, 0])
    theta = float(I["theta"][0, 0])
    xt = np.ascontiguousarray(I["x"].astype(np.float32).T)   # [512, 8192]
    shared = {
        "w1r": np.ascontiguousarray(I["w1"].astype(np.float32)),
        "b1": I["b1"].astype(np.float32).reshape(-1, 1),
        "w2h": np.ascontiguousarray(I["w2"].astype(np.float16)),
        "b2": I["b2"].astype(np.float32).reshape(-1, 1),
        "w3h": np.ascontiguousarray(I["w3"].astype(np.float16)),
        "b3": I["b3"].astype(np.float32).reshape(-1, 1),
        "gw1h": np.ascontiguousarray(I["gw1"].astype(np.float16)),
        "gb1": I["gb1"].astype(np.float32).reshape(-1, 1),
        "gw2h": np.ascontiguousarray(I["gw2"].astype(np.float16)),
        "gb2": I["gb2"].astype(np.float32).reshape(-1, 1),
        "lw1h": np.ascontiguousarray(I["lw1"].astype(np.float16)),
        "lb1": I["lb1"].astype(np.float32).reshape(-1, 1),
        "lw2h": np.ascontiguousarray(I["lw2"].astype(np.float16)),
        "lb2": I["lb2"].astype(np.float32).reshape(-1, 1),
    }
    in_maps = []
    for c in range(NCORES):
        m = dict(shared)
        m["xt"] = np.ascontiguousarray(xt[:, c * LOC:(c + 1) * LOC])
        in_maps.append(m)
    return t, theta, in_maps


def _execute(inputs, **run_kwargs):
    t, theta, in_maps = _prep_inputs(inputs)
    nc = _get_nc(t, theta)
    try:
        res = bass_utils.run_bass_kernel_spmd(
            nc, in_maps, core_ids=list(range(NCORES)), **run_kwargs)
    except ModuleNotFoundError:
        # NTFF trace hook unavailable in this container — run untraced
        os.environ["BASS_NEVER_TRACE"] = "1"
        run_kwargs.pop("trace", None)
        res = bass_utils.run_bass_kernel_spmd(
            nc, in_maps, core_ids=list(range(NCORES)), **run_kwargs)
    out = np.concatenate(
        [res.results[c]["outT"].T for c in range(NCORES)], axis=0)
    return np.ascontiguousarray(out.astype(np.float32)), res


def kernel(**inputs) -> np.ndarray:
    out, _ = _execute(inputs)
    return out


# revision 61
# speedup vs baseline: 1.0662x; 1.0170x over previous
"""Trainium2 Bass kernel for nn_DGraph_GAT (dense latent-graph GraphConv).

Strategy (8 NeuronCores, row-sharded over the 8192 nodes, 1024 nodes/core):
  - feature-major ("transposed") layouts everywhere: tensors are [feat, node]
  - per-core MLP encoder on the local 1024 nodes (layer1 float32r, 2-3 fp16)
  - AllGather of fp16 h and sq(h) (66KB/rank); X1 = h@gw1 and X2 = h1@gw2
    lhsT tiles are recomputed per core from the gathered fp16 data
  - cdist+sigmoid adjacency: an augmented K=34 fp16 matmul produces
    t^2*d2[j,i] directly in PSUM; ACT Sqrt (with +eps bias, clamps fp
    rounding negatives) evacuates PSUM -> fp16 A-buffer; one batched ACT
    Sigmoid pass turns it into A = sigmoid(t*(d+theta)) in place.
    A^T (column block, symmetric) stays fp16-resident in SBUF (16.8MB).
  - GraphConv layers: A^T block streamed through the PE as the moving
    operand against the small gathered X1/X2 matrices (PSUM-accumulated).
  - second tiny AllGather for X2 = h1@gw2, then the output MLP.
  - host side: shard/transpose inputs, gather per-core [16,1024] outputs.
"""

import os

import numpy as np

import concourse.bacc as bacc
import concourse.bass_utils as bass_utils
import concourse.mybir as mybir
import concourse.tile as tile
from concourse.tile_rust import add_dep_helper

F32 = mybir.dt.float32
F32R = mybir.dt.float32r
F16 = mybir.dt.float16
AF = mybir.ActivationFunctionType

NCORES = 8
N = 8192
LOC = N // NCORES          # 1024 nodes per core
JT = N // 128              # 64 j-tiles of 128 global nodes
IN_F, H_F, OUT_F = 512, 256, 32
NCLS = 16
INV = 1.0 / N
EPS_REL = 0.3              # sqrt clamp bias, as a fraction of t^2

# gather-1 flat layout (fp16 elements per rank)
G1_H = OUT_F * LOC         # 32768  h rows
G1_SQ = LOC                # 1024   sq row
G1_TOT = G1_H + G1_SQ      # 33792
G2_TOT = LOC * 16          # 16384  h1 rows (f n)


def _build(t: float, theta: float, sim: bool = False):
    tsq = t * t
    sgn = 1.0 if t >= 0 else -1.0
    nc = bacc.Bacc("TRN2", target_bir_lowering=False, debug=False,
                   enable_asserts=False,
                   num_devices=1 if sim else NCORES)

    def allgather(g_in, g_out, nelem):
        if sim:
            # cost-model build: stand in for the collective with local copies
            # into every rank segment (so unpack deps behave like the real AG)
            for r in range(NCORES):
                nc.sync.dma_start(
                    g_out[r * nelem:(r + 1) * nelem].rearrange(
                        "(o n) -> o n", o=1),
                    g_in[:].rearrange("(o n) -> o n", o=1))
        else:
            nc.gpsimd.collective_compute(
                "AllGather", mybir.AluOpType.bypass,
                replica_groups=[list(range(NCORES))],
                ins=[g_in.opt()], outs=[g_out.opt()])

    # ---- kernel I/O -----------------------------------------------------
    xt_d = nc.dram_tensor("xt", [IN_F, LOC], F32R, kind="ExternalInput")
    w1_d = nc.dram_tensor("w1r", [IN_F, H_F], F32R, kind="ExternalInput")
    b1_d = nc.dram_tensor("b1", [H_F, 1], F32, kind="ExternalInput")
    w2_d = nc.dram_tensor("w2h", [H_F, H_F], F16, kind="ExternalInput")
    b2_d = nc.dram_tensor("b2", [H_F, 1], F32, kind="ExternalInput")
    w3_d = nc.dram_tensor("w3h", [H_F, OUT_F], F16, kind="ExternalInput")
    b3_d = nc.dram_tensor("b3", [OUT_F, 1], F32, kind="ExternalInput")
    gw1_d = nc.dram_tensor("gw1h", [32, 16], F16, kind="ExternalInput")
    gb1_d = nc.dram_tensor("gb1", [16, 1], F32, kind="ExternalInput")
    gw2_d = nc.dram_tensor("gw2h", [16, 8], F16, kind="ExternalInput")
    gb2_d = nc.dram_tensor("gb2", [8, 1], F32, kind="ExternalInput")
    lw1_d = nc.dram_tensor("lw1h", [8, 16], F16, kind="ExternalInput")
    lb1_d = nc.dram_tensor("lb1", [16, 1], F32, kind="ExternalInput")
    lw2_d = nc.dram_tensor("lw2h", [16, 16], F16, kind="ExternalInput")
    lb2_d = nc.dram_tensor("lb2", [NCLS, 1], F32, kind="ExternalInput")
    out_d = nc.dram_tensor("outT", [NCLS, LOC], F32, kind="ExternalOutput")

    with tile.TileContext(nc) as tc:
        with (
            tc.tile_pool(name="dram", bufs=1, space="DRAM") as dram,
            tc.tile_pool(name="outer", bufs=1) as outer,
        ):
            # ---- persistent SBUF tensors -------------------------------
            X_big = outer.tile([128, JT * LOC], F16)      # A^T block / d
            hGT = outer.tile([34, N], F16)                # G lhsT (h,sq,1)
            rhs_G = outer.tile([34, LOC], F16)            # G moving operand
            X1_sb = outer.tile([128, JT * 16], F16)       # X1 lhsT tiles
            X2_sb = outer.tile([128, JT * 8], F16)        # X2 lhsT tiles
            hT3 = outer.tile([OUT_F, LOC], F32)           # local h (fp32)
            hb = outer.tile([OUT_F, LOC], F16)            # local h (fp16)
            sq16 = outer.tile([1, LOC], F16)
            h1T = outer.tile([16, LOC], F16)
            h2T = outer.tile([8, LOC], F16)
            h3T = outer.tile([16, LOC], F16)
            outT = outer.tile([NCLS, LOC], F32)
            gw1_sb = outer.tile([32, 16], F16)
            gb1_sb = outer.tile([16, 1], F32)
            gw2_sb = outer.tile([16, 8], F16)
            gb2_sb = outer.tile([8, 1], F32)
            lw1_sb = outer.tile([8, 16], F16)
            lb1_sb = outer.tile([16, 1], F32)
            lw2_sb = outer.tile([16, 16], F16)
            lb2_sb = outer.tile([NCLS, 1], F32)
            eps_sb = outer.tile([128, 1], F32)
            sigb_sb = outer.tile([128, 1], F32)
            ones32 = outer.tile([32, 1], F16)
            ones_row = outer.tile([1, LOC], F16)
            tsq_row = outer.tile([1, LOC], F16)

            warm_sb = outer.tile([1, 1], F32)
            nc.gpsimd.memset(warm_sb[:], 1.0)
            # preload the sqrt table set during the (ACT-idle) MLP phase;
            # Relu/Identity are filler funcs present in every set
            nc.scalar.activation(warm_sb[:], warm_sb[:], AF.Sqrt)
            nc.gpsimd.memset(eps_sb[:], tsq * EPS_REL)
            nc.gpsimd.memset(sigb_sb[:], t * theta)
            nc.gpsimd.memset(ones32[:], 1.0)
            nc.gpsimd.memset(ones_row[:], 1.0)
            nc.gpsimd.memset(tsq_row[:], tsq)

            g1i = dram.tile([G1_TOT], F16)
            g1o = dram.tile([NCORES * G1_TOT], F16)
            g2ai = dram.tile([G2_TOT // 2], F16)
            g2ao = dram.tile([NCORES * G2_TOT // 2], F16)
            g2bi = dram.tile([G2_TOT // 2], F16)
            g2bo = dram.tile([NCORES * G2_TOT // 2], F16)

            # ================= phase A: local MLP =======================
            with (
                tc.tile_pool(name="mlp", bufs=1) as mlp,
                tc.tile_pool(name="xts", bufs=3) as xts,
                tc.tile_pool(name="mlp_ps", bufs=4, space="PSUM") as mps,
            ):
                w1_sb = mlp.tile([128, 4 * H_F], F32R)
                w2_sb = mlp.tile([128, 2 * H_F], F16)
                w3_sb = mlp.tile([128, 2 * OUT_F], F16)
                b1_sb = mlp.tile([128, 2], F32)
                b2_sb = mlp.tile([128, 2], F32)
                b3_sb = mlp.tile([OUT_F, 1], F32)
                hT1a = mlp.tile([128, LOC], F16)
                hT1b = mlp.tile([128, LOC], F16)
                hT2a = mlp.tile([128, LOC], F16)
                hT2b = mlp.tile([128, LOC], F16)
                sqh = mlp.tile([OUT_F, LOC], F16)



                hT1 = (hT1a, hT1b)
                hT2 = (hT2a, hT2b)
                # layer 1: [512,256] @ xT, float32r; k-outer, 4 accumulators
                pm1 = [[mps.tile([128, 512], F32, tag="pm", name=f"pm1_{m}{n}")
                        for n in range(2)] for m in range(2)]
                for k in range(4):
                    nc.sync.dma_start(w1_sb[:, k * H_F:(k + 1) * H_F],
                                      w1_d[k * 128:(k + 1) * 128, :])
                    xt_k = xts.tile([128, LOC], F32R, tag="xtk")
                    nc.sync.dma_start(xt_k[:], xt_d[k * 128:(k + 1) * 128, :])
                    if k == 0:
                        for kk in range(2):
                            nc.sync.dma_start(b1_sb[:, kk:kk + 1],
                                              b1_d[kk * 128:(kk + 1) * 128, :])
                    if k == 1:
                        for kk in range(2):
                            nc.sync.dma_start(
                                w2_sb[:, kk * H_F:(kk + 1) * H_F],
                                w2_d[kk * 128:(kk + 1) * 128, :])
                            nc.sync.dma_start(b2_sb[:, kk:kk + 1],
                                              b2_d[kk * 128:(kk + 1) * 128, :])
                    if k == 2:
                        for kk in range(2):
                            nc.sync.dma_start(
                                w3_sb[:, kk * OUT_F:(kk + 1) * OUT_F],
                                w3_d[kk * 128:(kk + 1) * 128, :])
                        nc.sync.dma_start(b3_sb[:], b3_d[:])
                    for m in range(2):
                        for n in range(2):
                            nc.tensor.matmul(
                                pm1[m][n][:],
                                w1_sb[:, k * H_F + m * 128:k * H_F + (m + 1) * 128],
                                xt_k[:, n * 512:(n + 1) * 512],
                                start=(k == 0), stop=(k == 3))
                for m in range(2):
                    for n in range(2):
                        nc.scalar.activation(
                            hT1[m][:, n * 512:(n + 1) * 512], pm1[m][n][:],
                            AF.Relu, bias=b1_sb[:, m:m + 1], scale=1.0)
                # layer 2: fp16
                for m in range(2):
                    for n in range(2):
                        pm = mps.tile([128, 512], F32, tag="pm")
                        for k in range(2):
                            nc.tensor.matmul(
                                pm[:],
                                w2_sb[:, k * H_F + m * 128:k * H_F + (m + 1) * 128],
                                hT1[k][:, n * 512:(n + 1) * 512],
                                start=(k == 0), stop=(k == 1))
                        nc.scalar.activation(
                            hT2[m][:, n * 512:(n + 1) * 512], pm[:], AF.Relu,
                            bias=b2_sb[:, m:m + 1], scale=1.0)
                # layer 3: fp16 -> hT3 [32, 1024] fp32 (no relu)
                for n in range(2):
                    pm = mps.tile([OUT_F, 512], F32, tag="pm")
                    for k in range(2):
                        nc.tensor.matmul(
                            pm[:],
                            w3_sb[:, k * OUT_F:(k + 1) * OUT_F],
                            hT2[k][:, n * 512:(n + 1) * 512],
                            start=(k == 0), stop=(k == 1))
                    nc.scalar.activation(
                        hT3[:, n * 512:(n + 1) * 512], pm[:], AF.Identity,
                        bias=b3_sb[:], scale=1.0)

                # fp16 copy of h + sq = sum_f h16^2 (fp16 matmul w/ ones);
                # sq16 and the rhs_G row are written straight from PSUM
                nc.vector.tensor_copy(hb[:], hT3[:])
                nc.vector.tensor_mul(sqh[:], hb[:], hb[:])
                for n in range(2):
                    pm = mps.tile([1, 512], F32, tag="pm")
                    nc.tensor.matmul(pm[:], ones32[:],
                                     sqh[:, n * 512:(n + 1) * 512],
                                     start=True, stop=True)
                    nc.vector.tensor_copy(sq16[:, n * 512:(n + 1) * 512], pm[:])
                    nc.vector.tensor_scalar_mul(
                        rhs_G[32:33, n * 512:(n + 1) * 512], pm[:], tsq)

            # ================= gather 1 =================================
            nc.sync.dma_start(
                g1i[0:G1_H].rearrange("(f n) -> f n", f=OUT_F), hb[:])
            nc.sync.dma_start(
                g1i[G1_H:G1_H + G1_SQ].rearrange("(o n) -> o n", o=1), sq16[:])

            # moving operand rows: 0-31 = -2 t^2 h_loc (pairs h_glob),
            # 32 = t^2 sq_loc (pairs ones), 33 = t^2 (pairs sq_glob)
            # => psum = t^2 * d2[j, i]   (local data only — build pre-gather)
            nc.vector.tensor_scalar_mul(rhs_G[0:32, :], hT3[:], -2.0 * tsq)

            # static rows + tail weights: fill the collective-latency window
            nc.sync.dma_start(rhs_G[33:34, :], tsq_row[:])
            for r in range(NCORES):
                nc.sync.dma_start(hGT[32:33, r * LOC:(r + 1) * LOC],
                                  ones_row[:])
            nc.gpsimd.dma_start(gw1_sb[:], gw1_d[:])
            nc.gpsimd.dma_start(gb1_sb[:], gb1_d[:])
            nc.gpsimd.dma_start(gw2_sb[:], gw2_d[:])
            nc.gpsimd.dma_start(gb2_sb[:], gb2_d[:])
            nc.gpsimd.dma_start(lw1_sb[:], lw1_d[:])
            nc.gpsimd.dma_start(lb1_sb[:], lb1_d[:])
            nc.gpsimd.dma_start(lw2_sb[:], lw2_d[:])
            nc.gpsimd.dma_start(lb2_sb[:], lb2_d[:])

            allgather(g1i, g1o, G1_TOT)
            g1o_v = g1o[:].rearrange("(r q) -> r q", r=NCORES)
            for r in range(NCORES):
                nc.sync.dma_start(
                    hGT[0:OUT_F, r * LOC:(r + 1) * LOC],
                    g1o_v[r, 0:G1_H].rearrange("(f n) -> f n", f=OUT_F))
                nc.sync.dma_start(
                    hGT[33:34, r * LOC:(r + 1) * LOC],
                    g1o_v[r, G1_H:G1_H + G1_SQ].rearrange(
                        "(o n) -> o n", o=1))

            # ================= phase B: adjacency =======================
            sqrt_insts = []
            with tc.tile_pool(name="g_ps", bufs=2, space="PSUM") as gps:
                # two j-tiles per PSUM tile (4 banks x 2 bufs = all of PSUM)
                for jj in range(JT // 2):
                    pg = gps.tile([128, 2 * LOC], F32, tag="pg")
                    for half in range(4):
                        nc.tensor.matmul(
                            pg[:, half * 512:(half + 1) * 512],
                            hGT[:, 2 * jj * 128 + (half // 2) * 128:
                                2 * jj * 128 + (half // 2) * 128 + 128],
                            rhs_G[:, (half % 2) * 512:(half % 2) * 512 + 512],
                            start=True, stop=True, skip_group_check=True)
                    inst = nc.scalar.activation(
                        X_big[:, 2 * jj * LOC:(2 * jj + 2) * LOC], pg[:],
                        AF.Sqrt, bias=eps_sb[:], scale=1.0)
                    sqrt_insts.append(inst)

            with (
                tc.tile_pool(name="acc_ps", bufs=2, space="PSUM") as aps,
                tc.tile_pool(name="x2_ps", bufs=2, space="PSUM") as xps,
                tc.tile_pool(name="x1_ps", bufs=2, space="PSUM") as x1ps,
                tc.tile_pool(name="o2_ps", bufs=2, space="PSUM") as o2ps,
                tc.tile_pool(name="xts2", bufs=4) as xts2,
            ):
                # X1 lhsT tiles from gathered fp16 h — PE is idle during the
                # sigmoid window, so schedule these after the G matmuls
                for g in range(8):
                    px1 = x1ps.tile([128, 128], F32, tag="px1", name=f"px1_{g}")
                    for l in range(8):
                        j = g * 8 + l
                        nc.tensor.matmul(px1[:, l * 16:(l + 1) * 16],
                                         hGT[0:32, j * 128:(j + 1) * 128],
                                         gw1_sb[:], start=True, stop=True,
                                         skip_group_check=True)
                    nc.vector.tensor_copy(
                        X1_sb[:, g * 128:(g + 1) * 128], px1[:])

                # sigmoid + GraphConv1 + gather2, pipelined by node-half:
                # sweep half h of every A-tile, finish o1[h], relu, and fire
                # that half's h1 AllGather while the other half still runs.
                o1a = aps.tile([16, 512], F32, tag="acc")
                o1b = aps.tile([16, 512], F32, tag="acc")
                o1 = (o1a, o1b)
                o2a = o2ps.tile([8, 512], F32, tag="o2")
                o2b = o2ps.tile([8, 512], F32, tag="o2")
                o2 = (o2a, o2b)
                g2io = ((g2ai, g2ao), (g2bi, g2bo))

                def spmm2_tiles(tiles, start_j=None, stop_j=None):
                    insts = []
                    for j in tiles:
                        for ih in range(2):
                            insts.append(nc.tensor.matmul(
                                o2[ih][:], X2_sb[:, j * 8:(j + 1) * 8],
                                X_big[:, j * LOC + ih * 512:
                                      j * LOC + ih * 512 + 512],
                                start=(j == start_j), stop=(j == stop_j)))
                    return insts

                # group-a j-tiles (first 4 of each rank block), chunk-ordered
                ga_tiles = [8 * g + l for g in range(8)
                            for l in range(8) if (8 * g + l) % 8 < 4]

                def sig_spmm1_gather(h):
                    last_i2 = [None]
                    for g in range(8):
                        ap4 = X_big[:, g * 8192:(g + 1) * 8192].rearrange(
                            "p (l s n) -> p l s n", l=8, s=2)[
                            :, :, h:h + 1, :]
                        inst = nc.scalar.activation(ap4, ap4, AF.Sigmoid,
                                                    bias=sigb_sb[:], scale=sgn)
                        add_dep_helper(inst.ins, sqrt_insts[-1].ins,
                                       sync=False,
                                       reason="batch ACT table sets")
                        prev = None
                        for j in range(8 * g, 8 * g + 8):
                            prev = nc.tensor.matmul(
                                o1[h][:], X1_sb[:, j * 16:(j + 1) * 16],
                                X_big[:, j * LOC + h * 512:
                                      j * LOC + h * 512 + 512],
                                start=(j == 0), stop=(j == JT - 1))
                            if last_i2[0] is not None:
                                add_dep_helper(prev.ins, last_i2[0].ins,
                                               sync=False,
                                               reason="pin interleave order")
                                last_i2[0] = None
                        if h == 1 and g >= 3:
                            # fill PE idle slots of the sigmoid-b sweep with
                            # group-a SpMM2 tiles (X2a landed ~3 chunks ago);
                            # pin the alternation (scheduler would sink these)
                            gg = g - 3
                            ii = spmm2_tiles(ga_tiles[4 * gg:4 * gg + 4],
                                             start_j=ga_tiles[0])
                            add_dep_helper(ii[0].ins, prev.ins, sync=False,
                                           reason="pin spmm2a interleave")
                            last_i2[0] = ii[-1]
                    nc.scalar.activation(
                        h1T[:, h * 512:(h + 1) * 512], o1[h][:], AF.Relu,
                        bias=gb1_sb[:], scale=INV)
                    nc.sync.dma_start(
                        g2io[h][0][:].rearrange("(f n) -> f n", f=16),
                        h1T[:, h * 512:(h + 1) * 512])
                    allgather(g2io[h][0], g2io[h][1], G2_TOT // 2)

                def x2_prep(h):
                    for r in range(NCORES):
                        h1r = xts2.tile([16, 512], F16, tag="h1r",
                                        name=f"h1r_{h}_{r}")
                        nc.sync.dma_start(
                            h1r[:],
                            g2io[h][1][r * (G2_TOT // 2):
                                       (r + 1) * (G2_TOT // 2)].rearrange(
                                "(f n) -> f n", f=16))
                        px = xps.tile([128, 32], F32, tag="px2",
                                      name=f"px2_{h}_{r}")
                        for ll in range(4):
                            nc.tensor.matmul(px[:, ll * 8:(ll + 1) * 8],
                                             h1r[:, ll * 128:(ll + 1) * 128],
                                             gw2_sb[:], start=True, stop=True,
                                             skip_group_check=True)
                        nc.vector.tensor_copy(
                            X2_sb[:, r * 64 + h * 32:r * 64 + h * 32 + 32],
                            px[:])

                sig_spmm1_gather(0)
                x2_prep(0)              # half-a X2 ready during half-b sweep
                sig_spmm1_gather(1)     # interleaves 20 of 32 group-a tiles
                spmm2_tiles(ga_tiles[20:])      # remaining group-a tiles
                # half-b: unpack + X2 tiles + SpMM2 per rank, pipelined
                gb_last = NCORES * 8 - 1 - 0    # j = 63 is in group-b
                for r in range(NCORES):
                    h1r = xts2.tile([16, 512], F16, tag="h1r",
                                    name=f"h1rb_{r}")
                    nc.sync.dma_start(
                        h1r[:],
                        g2bo[r * (G2_TOT // 2):
                             (r + 1) * (G2_TOT // 2)].rearrange(
                            "(f n) -> f n", f=16))
                    px = xps.tile([128, 32], F32, tag="px2", name=f"px2b_{r}")
                    for ll in range(4):
                        nc.tensor.matmul(px[:, ll * 8:(ll + 1) * 8],
                                         h1r[:, ll * 128:(ll + 1) * 128],
                                         gw2_sb[:], start=True, stop=True,
                                         skip_group_check=True)
                    nc.vector.tensor_copy(
                        X2_sb[:, r * 64 + 32:r * 64 + 64], px[:])
                    spmm2_tiles([r * 8 + 4 + ll for ll in range(4)],
                                stop_j=gb_last)
                for ih in range(2):
                    nc.scalar.activation(
                        h2T[:, ih * 512:(ih + 1) * 512], o2[ih][:], AF.Relu,
                        bias=gb2_sb[:], scale=INV)

                # ---- output MLP ----------------------------------------
                for n in range(2):
                    ph = aps.tile([16, 512], F32, tag="acc")
                    nc.tensor.matmul(ph[:], lw1_sb[:],
                                     h2T[:, n * 512:(n + 1) * 512],
                                     start=True, stop=True)
                    nc.scalar.activation(
                        h3T[:, n * 512:(n + 1) * 512], ph[:], AF.Relu,
                        bias=lb1_sb[:], scale=1.0)
                for n in range(2):
                    po = aps.tile([NCLS, 512], F32, tag="acc")
                    nc.tensor.matmul(po[:], lw2_sb[:],
                                     h3T[:, n * 512:(n + 1) * 512],
                                     start=True, stop=True)
                    nc.scalar.activation(
                        outT[:, n * 512:(n + 1) * 512], po[:], AF.Identity,
                        bias=lb2_sb[:], scale=1.0)
                nc.sync.dma_start(out_d[:], outT[:])

    nc.compile()
    return nc


_CACHE = {}


def _get_nc(t: float, theta: float):
    key = (t, theta)
    if key not in _CACHE:
        _CACHE[key] = _build(t, theta)
    return _CACHE[key]


def _prep_inputs(inputs):
    I = {k: np.asarray(v) for k, v in inputs.items()}
    t = float(I["t"][0, 0])
    theta = float(I["theta"][0, 0])
    xt = np.ascontiguousarray(I["x"].astype(np.float32).T)   # [512, 8192]
    shared = {
        "w1r": np.ascontiguousarray(I["w1"].astype(np.float32)),
        "b1": I["b1"].astype(np.float32).reshape(-1, 1),
        "w2h": np.ascontiguousarray(I["w2"].astype(np.float16)),
        "b2": I["b2"].astype(np.float32).reshape(-1, 1),
        "w3h": np.ascontiguousarray(I["w3"].astype(np.float16)),
        "b3": I["b3"].astype(np.float32).reshape(-1, 1),
        "gw1h": np.ascontiguousarray(I["gw1"].astype(np.float16)),
        "gb1": I["gb1"].astype(np.float32).reshape(-1, 1),
        "gw2h": np.ascontiguousarray(I["gw2"].astype(np.float16)),
        "gb2": I["gb2"].astype(np.float32).reshape(-1, 1),
        "lw1h": np.ascontiguousarray(I["lw1"].astype(np.float16)),
        "lb1": I["lb1"].astype(np.float32).reshape(-1, 1),
        "lw2h": np.ascontiguousarray(I["lw2"].astype(np.float16)),
        "lb2": I["lb2"].astype(np.float32).reshape(-1, 1),
    }
    in_maps = []
    for c in range(NCORES):
        m = dict(shared)
        m["xt"] = np.ascontiguousarray(xt[:, c * LOC:(c + 1) * LOC])
        in_maps.append(m)
    return t, theta, in_maps


def _execute(inputs, **run_kwargs):
    t, theta, in_maps = _prep_inputs(inputs)
    nc = _get_nc(t, theta)
    try:
        res = bass_utils.run_bass_kernel_spmd(
            nc, in_maps, core_ids=list(range(NCORES)), **run_kwargs)
    except ModuleNotFoundError:
        # NTFF trace hook unavailable in this container — run untraced
        os.environ["BASS_NEVER_TRACE"] = "1"
        run_kwargs.pop("trace", None)
        res = bass_utils.run_bass_kernel_spmd(
            nc, in_maps, core_ids=list(range(NCORES)), **run_kwargs)
    out = np.concatenate(
        [res.results[c]["outT"].T for c in range(NCORES)], axis=0)
    return np.ascontiguousarray(out.astype(np.float32)), res


def kernel(**inputs) -> np.ndarray:
    out, _ = _execute(inputs)
    return out


# revision 66
# speedup vs baseline: 1.0668x; 1.0005x over previous
"""Trainium2 Bass kernel for nn_DGraph_GAT (dense latent-graph GraphConv).

Strategy (8 NeuronCores, row-sharded over the 8192 nodes, 1024 nodes/core):
  - feature-major ("transposed") layouts everywhere: tensors are [feat, node]
  - per-core MLP encoder on the local 1024 nodes (layer1 float32r, 2-3 fp16)
  - AllGather of fp16 h and sq(h) (66KB/rank); X1 = h@gw1 and X2 = h1@gw2
    lhsT tiles are recomputed per core from the gathered fp16 data
  - cdist+sigmoid adjacency: an augmented K=34 fp16 matmul produces
    t^2*d2[j,i] directly in PSUM; ACT Sqrt (with +eps bias, clamps fp
    rounding negatives) evacuates PSUM -> fp16 A-buffer; one batched ACT
    Sigmoid pass turns it into A = sigmoid(t*(d+theta)) in place.
    A^T (column block, symmetric) stays fp16-resident in SBUF (16.8MB).
  - GraphConv layers: A^T block streamed through the PE as the moving
    operand against the small gathered X1/X2 matrices (PSUM-accumulated).
  - second tiny AllGather for X2 = h1@gw2, then the output MLP.
  - host side: shard/transpose inputs, gather per-core [16,1024] outputs.
"""

import os

import numpy as np

import concourse.bacc as bacc
import concourse.bass_utils as bass_utils
import concourse.mybir as mybir
import concourse.tile as tile
from concourse.tile_rust import add_dep_helper

F32 = mybir.dt.float32
F32R = mybir.dt.float32r
F16 = mybir.dt.float16
AF = mybir.ActivationFunctionType

NCORES = 8
N = 8192
LOC = N // NCORES          # 1024 nodes per core
JT = N // 128              # 64 j-tiles of 128 global nodes
IN_F, H_F, OUT_F = 512, 256, 32
NCLS = 16
INV = 1.0 / N
EPS_REL = 0.3              # sqrt clamp bias, as a fraction of t^2

# gather-1 flat layout (fp16 elements per rank)
G1_H = OUT_F * LOC         # 32768  h rows
G1_SQ = LOC                # 1024   sq row
G1_TOT = G1_H + G1_SQ      # 33792
G2_TOT = LOC * 16          # 16384  h1 rows (f n)


def _build(t: float, theta: float, sim: bool = False):
    tsq = t * t
    sgn = 1.0 if t >= 0 else -1.0
    nc = bacc.Bacc("TRN2", target_bir_lowering=False, debug=False,
                   enable_asserts=False,
                   num_devices=1 if sim else NCORES)

    def allgather(g_in, g_out, nelem):
        if sim:
            # cost-model build: stand in for the collective with local copies
            # into every rank segment (so unpack deps behave like the real AG)
            for r in range(NCORES):
                nc.sync.dma_start(
                    g_out[r * nelem:(r + 1) * nelem].rearrange(
                        "(o n) -> o n", o=1),
                    g_in[:].rearrange("(o n) -> o n", o=1))
        else:
            nc.gpsimd.collective_compute(
                "AllGather", mybir.AluOpType.bypass,
                replica_groups=[list(range(NCORES))],
                ins=[g_in.opt()], outs=[g_out.opt()])

    # ---- kernel I/O -----------------------------------------------------
    xt_d = nc.dram_tensor("xt", [IN_F, LOC], F32R, kind="ExternalInput")
    w1_d = nc.dram_tensor("w1r", [IN_F, H_F], F32R, kind="ExternalInput")
    b1_d = nc.dram_tensor("b1", [H_F, 1], F32, kind="ExternalInput")
    w2_d = nc.dram_tensor("w2h", [H_F, H_F], F16, kind="ExternalInput")
    b2_d = nc.dram_tensor("b2", [H_F, 1], F32, kind="ExternalInput")
    w3_d = nc.dram_tensor("w3h", [H_F, OUT_F], F16, kind="ExternalInput")
    b3_d = nc.dram_tensor("b3", [OUT_F, 1], F32, kind="ExternalInput")
    gw1_d = nc.dram_tensor("gw1h", [32, 16], F16, kind="ExternalInput")
    gb1_d = nc.dram_tensor("gb1", [16, 1], F32, kind="ExternalInput")
    gw2_d = nc.dram_tensor("gw2h", [16, 8], F16, kind="ExternalInput")
    gb2_d = nc.dram_tensor("gb2", [8, 1], F32, kind="ExternalInput")
    lw1_d = nc.dram_tensor("lw1h", [8, 16], F16, kind="ExternalInput")
    lb1_d = nc.dram_tensor("lb1", [16, 1], F32, kind="ExternalInput")
    lw2_d = nc.dram_tensor("lw2h", [16, 16], F16, kind="ExternalInput")
    lb2_d = nc.dram_tensor("lb2", [NCLS, 1], F32, kind="ExternalInput")
    out_d = nc.dram_tensor("outT", [NCLS, LOC], F32, kind="ExternalOutput")

    with tile.TileContext(nc) as tc:
        with (
            tc.tile_pool(name="dram", bufs=1, space="DRAM") as dram,
            tc.tile_pool(name="outer", bufs=1) as outer,
        ):
            # ---- persistent SBUF tensors -------------------------------
            X_big = outer.tile([128, JT * LOC], F16)      # A^T block / d
            hGT = outer.tile([34, N], F16)                # G lhsT (h,sq,1)
            rhs_G = outer.tile([34, LOC], F16)            # G moving operand
            X1_sb = outer.tile([128, JT * 16], F16)       # X1 lhsT tiles
            X2_sb = outer.tile([128, JT * 8], F16)        # X2 lhsT tiles
            hT3 = outer.tile([OUT_F, LOC], F32)           # local h (fp32)
            hb = outer.tile([OUT_F, LOC], F16)            # local h (fp16)
            sq16 = outer.tile([1, LOC], F16)
            h1T = outer.tile([16, LOC], F16)
            h2T = outer.tile([8, LOC], F16)
            h3T = outer.tile([16, LOC], F16)
            outT = outer.tile([NCLS, LOC], F32)
            gw1_sb = outer.tile([32, 16], F16)
            gb1_sb = outer.tile([16, 1], F32)
            gw2_sb = outer.tile([16, 8], F16)
            gb2_sb = outer.tile([8, 1], F32)
            lw1_sb = outer.tile([8, 16], F16)
            lb1_sb = outer.tile([16, 1], F32)
            lw2_sb = outer.tile([16, 16], F16)
            lb2_sb = outer.tile([NCLS, 1], F32)
            eps_sb = outer.tile([128, 1], F32)
            sigb_sb = outer.tile([128, 1], F32)
            ones32 = outer.tile([32, 1], F16)
            ones_row = outer.tile([1, LOC], F16)
            tsq_row = outer.tile([1, LOC], F16)

            warm_sb = outer.tile([1, 1], F32)
            nc.gpsimd.memset(warm_sb[:], 1.0)
            # preload the sqrt table set during the (ACT-idle) MLP phase;
            # Relu/Identity are filler funcs present in every set
            nc.scalar.activation(warm_sb[:], warm_sb[:], AF.Sqrt)
            nc.gpsimd.memset(eps_sb[:], tsq * EPS_REL)
            nc.gpsimd.memset(sigb_sb[:], t * theta)
            nc.gpsimd.memset(ones32[:], 1.0)
            nc.gpsimd.memset(ones_row[:], 1.0)
            nc.gpsimd.memset(tsq_row[:], tsq)

            g1i = dram.tile([G1_TOT], F16)
            g1o = dram.tile([NCORES * G1_TOT], F16)
            g2ai = dram.tile([G2_TOT // 2], F16)
            g2ao = dram.tile([NCORES * G2_TOT // 2], F16)
            g2bi = dram.tile([G2_TOT // 2], F16)
            g2bo = dram.tile([NCORES * G2_TOT // 2], F16)

            # ================= phase A: local MLP =======================
            with (
                tc.tile_pool(name="mlp", bufs=1) as mlp,
                tc.tile_pool(name="xts", bufs=3) as xts,
                tc.tile_pool(name="mlp_ps", bufs=4, space="PSUM") as mps,
            ):
                w1_sb = mlp.tile([128, 4 * H_F], F32R)
                w2_sb = mlp.tile([128, 2 * H_F], F16)
                w3_sb = mlp.tile([128, 2 * OUT_F], F16)
                b1_sb = mlp.tile([128, 2], F32)
                b2_sb = mlp.tile([128, 2], F32)
                b3_sb = mlp.tile([OUT_F, 1], F32)
                hT1a = mlp.tile([128, LOC], F16)
                hT1b = mlp.tile([128, LOC], F16)
                hT2a = mlp.tile([128, LOC], F16)
                hT2b = mlp.tile([128, LOC], F16)
                sqh = mlp.tile([OUT_F, LOC], F16)



                hT1 = (hT1a, hT1b)
                hT2 = (hT2a, hT2b)
                # layer 1: [512,256] @ xT, float32r; k-outer, 4 accumulators
                pm1 = [[mps.tile([128, 512], F32, tag="pm", name=f"pm1_{m}{n}")
                        for n in range(2)] for m in range(2)]
                for k in range(4):
                    nc.sync.dma_start(w1_sb[:, k * H_F:(k + 1) * H_F],
                                      w1_d[k * 128:(k + 1) * 128, :])
                    xt_k = xts.tile([128, LOC], F32R, tag="xtk")
                    nc.sync.dma_start(xt_k[:], xt_d[k * 128:(k + 1) * 128, :])
                    if k == 0:
                        for kk in range(2):
                            nc.sync.dma_start(b1_sb[:, kk:kk + 1],
                                              b1_d[kk * 128:(kk + 1) * 128, :])
                    if k == 1:
                        for kk in range(2):
                            nc.sync.dma_start(
                                w2_sb[:, kk * H_F:(kk + 1) * H_F],
                                w2_d[kk * 128:(kk + 1) * 128, :])
                            nc.sync.dma_start(b2_sb[:, kk:kk + 1],
                                              b2_d[kk * 128:(kk + 1) * 128, :])
                    if k == 2:
                        for kk in range(2):
                            nc.sync.dma_start(
                                w3_sb[:, kk * OUT_F:(kk + 1) * OUT_F],
                                w3_d[kk * 128:(kk + 1) * 128, :])
                        nc.sync.dma_start(b3_sb[:], b3_d[:])
                    for m in range(2):
                        for n in range(2):
                            nc.tensor.matmul(
                                pm1[m][n][:],
                                w1_sb[:, k * H_F + m * 128:k * H_F + (m + 1) * 128],
                                xt_k[:, n * 512:(n + 1) * 512],
                                start=(k == 0), stop=(k == 3))
                for m in range(2):
                    for n in range(2):
                        nc.scalar.activation(
                            hT1[m][:, n * 512:(n + 1) * 512], pm1[m][n][:],
                            AF.Relu, bias=b1_sb[:, m:m + 1], scale=1.0)
                # layer 2: fp16
                for m in range(2):
                    for n in range(2):
                        pm = mps.tile([128, 512], F32, tag="pm")
                        for k in range(2):
                            nc.tensor.matmul(
                                pm[:],
                                w2_sb[:, k * H_F + m * 128:k * H_F + (m + 1) * 128],
                                hT1[k][:, n * 512:(n + 1) * 512],
                                start=(k == 0), stop=(k == 1))
                        nc.scalar.activation(
                            hT2[m][:, n * 512:(n + 1) * 512], pm[:], AF.Relu,
                            bias=b2_sb[:, m:m + 1], scale=1.0)
                # layer 3: fp16 -> hT3 [32, 1024] fp32 (no relu)
                for n in range(2):
                    pm = mps.tile([OUT_F, 512], F32, tag="pm")
                    for k in range(2):
                        nc.tensor.matmul(
                            pm[:],
                            w3_sb[:, k * OUT_F:(k + 1) * OUT_F],
                            hT2[k][:, n * 512:(n + 1) * 512],
                            start=(k == 0), stop=(k == 1))
                    nc.scalar.activation(
                        hT3[:, n * 512:(n + 1) * 512], pm[:], AF.Identity,
                        bias=b3_sb[:], scale=1.0)

                # fp16 copy of h + sq = sum_f h16^2 (fp16 matmul w/ ones);
                # per node-half so the chain starts on hT3's first half;
                # sq16 and the rhs_G row are written straight from PSUM
                for n in range(2):
                    sl = slice(n * 512, (n + 1) * 512)
                    nc.vector.tensor_copy(hb[:, sl], hT3[:, sl])
                    nc.vector.tensor_mul(sqh[:, sl], hb[:, sl], hb[:, sl])
                    pm = mps.tile([1, 512], F32, tag="pm")
                    nc.tensor.matmul(pm[:], ones32[:], sqh[:, sl],
                                     start=True, stop=True)
                    nc.vector.tensor_copy(sq16[:, sl], pm[:])
                    nc.vector.tensor_scalar_mul(rhs_G[32:33, sl], pm[:], tsq)

            # ================= gather 1 =================================
            nc.sync.dma_start(
                g1i[0:G1_H].rearrange("(f n) -> f n", f=OUT_F), hb[:])
            nc.sync.dma_start(
                g1i[G1_H:G1_H + G1_SQ].rearrange("(o n) -> o n", o=1), sq16[:])

            # moving operand rows: 0-31 = -2 t^2 h_loc (pairs h_glob),
            # 32 = t^2 sq_loc (pairs ones), 33 = t^2 (pairs sq_glob)
            # => psum = t^2 * d2[j, i]   (local data only — build pre-gather)
            nc.vector.tensor_scalar_mul(rhs_G[0:32, :], hT3[:], -2.0 * tsq)

            # static rows + tail weights: fill the collective-latency window
            nc.sync.dma_start(rhs_G[33:34, :], tsq_row[:])
            for r in range(NCORES):
                nc.sync.dma_start(hGT[32:33, r * LOC:(r + 1) * LOC],
                                  ones_row[:])
            nc.gpsimd.dma_start(gw1_sb[:], gw1_d[:])
            nc.gpsimd.dma_start(gb1_sb[:], gb1_d[:])
            nc.gpsimd.dma_start(gw2_sb[:], gw2_d[:])
            nc.gpsimd.dma_start(gb2_sb[:], gb2_d[:])
            nc.gpsimd.dma_start(lw1_sb[:], lw1_d[:])
            nc.gpsimd.dma_start(lb1_sb[:], lb1_d[:])
            nc.gpsimd.dma_start(lw2_sb[:], lw2_d[:])
            nc.gpsimd.dma_start(lb2_sb[:], lb2_d[:])

            allgather(g1i, g1o, G1_TOT)
            g1o_v = g1o[:].rearrange("(r q) -> r q", r=NCORES)
            for r in range(NCORES):
                nc.sync.dma_start(
                    hGT[0:OUT_F, r * LOC:(r + 1) * LOC],
                    g1o_v[r, 0:G1_H].rearrange("(f n) -> f n", f=OUT_F))
                nc.sync.dma_start(
                    hGT[33:34, r * LOC:(r + 1) * LOC],
                    g1o_v[r, G1_H:G1_H + G1_SQ].rearrange(
                        "(o n) -> o n", o=1))

            # ================= phase B: adjacency =======================
            sqrt_insts = []
            with tc.tile_pool(name="g_ps", bufs=2, space="PSUM") as gps:
                # two j-tiles per PSUM tile (4 banks x 2 bufs = all of PSUM)
                for jj in range(JT // 2):
                    pg = gps.tile([128, 2 * LOC], F32, tag="pg")
                    for half in range(4):
                        nc.tensor.matmul(
                            pg[:, half * 512:(half + 1) * 512],
                            hGT[:, 2 * jj * 128 + (half // 2) * 128:
                                2 * jj * 128 + (half // 2) * 128 + 128],
                            rhs_G[:, (half % 2) * 512:(half % 2) * 512 + 512],
                            start=True, stop=True, skip_group_check=True)
                    inst = nc.scalar.activation(
                        X_big[:, 2 * jj * LOC:(2 * jj + 2) * LOC], pg[:],
                        AF.Sqrt, bias=eps_sb[:], scale=1.0)
                    sqrt_insts.append(inst)

            with (
                tc.tile_pool(name="acc_ps", bufs=2, space="PSUM") as aps,
                tc.tile_pool(name="x2_ps", bufs=2, space="PSUM") as xps,
                tc.tile_pool(name="x1_ps", bufs=2, space="PSUM") as x1ps,
                tc.tile_pool(name="o2_ps", bufs=2, space="PSUM") as o2ps,
                tc.tile_pool(name="xts2", bufs=4) as xts2,
            ):
                # X1 lhsT tiles from gathered fp16 h — PE is idle during the
                # sigmoid window, so schedule these after the G matmuls
                for g in range(8):
                    px1 = x1ps.tile([128, 128], F32, tag="px1", name=f"px1_{g}")
                    for l in range(8):
                        j = g * 8 + l
                        nc.tensor.matmul(px1[:, l * 16:(l + 1) * 16],
                                         hGT[0:32, j * 128:(j + 1) * 128],
                                         gw1_sb[:], start=True, stop=True,
                                         skip_group_check=True)
                    nc.vector.tensor_copy(
                        X1_sb[:, g * 128:(g + 1) * 128], px1[:])

                # sigmoid + GraphConv1 + gather2, pipelined by node-half:
                # sweep half h of every A-tile, finish o1[h], relu, and fire
                # that half's h1 AllGather while the other half still runs.
                o1a = aps.tile([16, 512], F32, tag="acc")
                o1b = aps.tile([16, 512], F32, tag="acc")
                o1 = (o1a, o1b)
                o2a = o2ps.tile([8, 512], F32, tag="o2")
                o2b = o2ps.tile([8, 512], F32, tag="o2")
                o2 = (o2a, o2b)
                g2io = ((g2ai, g2ao), (g2bi, g2bo))

                def spmm2_tiles(tiles, start_j=None, stop_j=None):
                    insts = []
                    for j in tiles:
                        for ih in range(2):
                            insts.append(nc.tensor.matmul(
                                o2[ih][:], X2_sb[:, j * 8:(j + 1) * 8],
                                X_big[:, j * LOC + ih * 512:
                                      j * LOC + ih * 512 + 512],
                                start=(j == start_j), stop=(j == stop_j)))
                    return insts

                # group-a j-tiles (first 4 of each rank block), chunk-ordered
                ga_tiles = [8 * g + l for g in range(8)
                            for l in range(8) if (8 * g + l) % 8 < 4]

                inter_idx = [0]

                def sig_spmm1_gather(h):
                    last_i2 = [None]
                    # sub-chunks: (tile_lo, n_tiles); final group of sweep-b
                    # split in half so o1's stop-matmul lands ~2us earlier
                    chunks = [(8 * g, 8) for g in range(7)]
                    if h == 1:
                        chunks += [(56, 4), (60, 4)]
                    else:
                        chunks += [(56, 8)]
                    for g, (lo, nt) in enumerate(chunks):
                        ap4 = X_big[:, lo * LOC:(lo + nt) * LOC].rearrange(
                            "p (l s n) -> p l s n", l=nt, s=2)[
                            :, :, h:h + 1, :]
                        inst = nc.scalar.activation(ap4, ap4, AF.Sigmoid,
                                                    bias=sigb_sb[:], scale=sgn)
                        add_dep_helper(inst.ins, sqrt_insts[-1].ins,
                                       sync=False,
                                       reason="batch ACT table sets")
                        prev = None
                        for j in range(lo, lo + nt):
                            prev = nc.tensor.matmul(
                                o1[h][:], X1_sb[:, j * 16:(j + 1) * 16],
                                X_big[:, j * LOC + h * 512:
                                      j * LOC + h * 512 + 512],
                                start=(j == 0), stop=(j == JT - 1))
                            if last_i2[0] is not None:
                                add_dep_helper(prev.ins, last_i2[0].ins,
                                               sync=False,
                                               reason="pin interleave order")
                                last_i2[0] = None
                        if h == 1 and g >= 3 and nt == 8:
                            # fill PE idle slots of the sigmoid-b sweep with
                            # group-a SpMM2 tiles (X2a landed ~3 chunks ago);
                            # pin the alternation (scheduler would sink these)
                            take = ga_tiles[inter_idx[0]:inter_idx[0] + 4]
                            inter_idx[0] += len(take)
                            if take:
                                ii = spmm2_tiles(take, start_j=ga_tiles[0])
                                add_dep_helper(ii[0].ins, prev.ins,
                                               sync=False,
                                               reason="pin spmm2a interleave")
                                last_i2[0] = ii[-1]
                    nc.scalar.activation(
                        h1T[:, h * 512:(h + 1) * 512], o1[h][:], AF.Relu,
                        bias=gb1_sb[:], scale=INV)
                    nc.sync.dma_start(
                        g2io[h][0][:].rearrange("(f n) -> f n", f=16),
                        h1T[:, h * 512:(h + 1) * 512])
                    allgather(g2io[h][0], g2io[h][1], G2_TOT // 2)

                def x2_prep(h):
                    for r in range(NCORES):
                        h1r = xts2.tile([16, 512], F16, tag="h1r",
                                        name=f"h1r_{h}_{r}")
                        nc.sync.dma_start(
                            h1r[:],
                            g2io[h][1][r * (G2_TOT // 2):
                                       (r + 1) * (G2_TOT // 2)].rearrange(
                                "(f n) -> f n", f=16))
                        px = xps.tile([128, 32], F32, tag="px2",
                                      name=f"px2_{h}_{r}")
                        for ll in range(4):
                            nc.tensor.matmul(px[:, ll * 8:(ll + 1) * 8],
                                             h1r[:, ll * 128:(ll + 1) * 128],
                                             gw2_sb[:], start=True, stop=True,
                                             skip_group_check=True)
                        nc.vector.tensor_copy(
                            X2_sb[:, r * 64 + h * 32:r * 64 + h * 32 + 32],
                            px[:])

                sig_spmm1_gather(0)
                x2_prep(0)              # half-a X2 ready during half-b sweep
                sig_spmm1_gather(1)     # interleaves 20 of 32 group-a tiles
                spmm2_tiles(ga_tiles[inter_idx[0]:])  # rest of group a
                # half-b: unpack + X2 tiles + SpMM2 per rank, pipelined
                gb_last = NCORES * 8 - 1 - 0    # j = 63 is in group-b
                for r in range(NCORES):
                    h1r = xts2.tile([16, 512], F16, tag="h1r",
                                    name=f"h1rb_{r}")
                    nc.sync.dma_start(
                        h1r[:],
                        g2bo[r * (G2_TOT // 2):
                             (r + 1) * (G2_TOT // 2)].rearrange(
                            "(f n) -> f n", f=16))
                    px = xps.tile([128, 32], F32, tag="px2", name=f"px2b_{r}")
                    for ll in range(4):
                        nc.tensor.matmul(px[:, ll * 8:(ll + 1) * 8],
                                         h1r[:, ll * 128:(ll + 1) * 128],
                                         gw2_sb[:], start=True, stop=True,
                                         skip_group_check=True)
                    nc.vector.tensor_copy(
                        X2_sb[:, r * 64 + 32:r * 64 + 64], px[:])
                    spmm2_tiles([r * 8 + 4 + ll for ll in range(4)],
                                stop_j=gb_last)
                for ih in range(2):
                    nc.scalar.activation(
                        h2T[:, ih * 512:(ih + 1) * 512], o2[ih][:], AF.Relu,
                        bias=gb2_sb[:], scale=INV)

                # ---- output MLP ----------------------------------------
                for n in range(2):
                    ph = aps.tile([16, 512], F32, tag="acc")
                    nc.tensor.matmul(ph[:], lw1_sb[:],
                                     h2T[:, n * 512:(n + 1) * 512],
                                     start=True, stop=True)
                    nc.scalar.activation(
                        h3T[:, n * 512:(n + 1) * 512], ph[:], AF.Relu,
                        bias=lb1_sb[:], scale=1.0)
                for n in range(2):
                    po = aps.tile([NCLS, 512], F32, tag="acc")
                    nc.tensor.matmul(po[:], lw2_sb[:],
                                     h3T[:, n * 512:(n + 1) * 512],
                                     start=True, stop=True)
                    nc.scalar.activation(
                        outT[:, n * 512:(n + 1) * 512], po[:], AF.Identity,
                        bias=lb2_sb[:], scale=1.0)
                nc.sync.dma_start(out_d[:], outT[:])

    nc.compile()
    return nc


_CACHE = {}


def _get_nc(t: float, theta: float):
    key = (t, theta)
    if key not in _CACHE:
        _CACHE[key] = _build(t, theta)
    return _CACHE[key]


def _prep_inputs(inputs):
    I = {k: np.asarray(v) for k, v in inputs.items()}
    t = float(I["t"][0, 0])
    theta = float(I["theta"][0, 0])
    xt = np.ascontiguousarray(I["x"].astype(np.float32).T)   # [512, 8192]
    shared = {
        "w1r": np.ascontiguousarray(I["w1"].astype(np.float32)),
        "b1": I["b1"].astype(np.float32).reshape(-1, 1),
        "w2h": np.ascontiguousarray(I["w2"].astype(np.float16)),
        "b2": I["b2"].astype(np.float32).reshape(-1, 1),
        "w3h": np.ascontiguousarray(I["w3"].astype(np.float16)),
        "b3": I["b3"].astype(np.float32).reshape(-1, 1),
        "gw1h": np.ascontiguousarray(I["gw1"].astype(np.float16)),
        "gb1": I["gb1"].astype(np.float32).reshape(-1, 1),
        "gw2h": np.ascontiguousarray(I["gw2"].astype(np.float16)),
        "gb2": I["gb2"].astype(np.float32).reshape(-1, 1),
        "lw1h": np.ascontiguousarray(I["lw1"].astype(np.float16)),
        "lb1": I["lb1"].astype(np.float32).reshape(-1, 1),
        "lw2h": np.ascontiguousarray(I["lw2"].astype(np.float16)),
        "lb2": I["lb2"].astype(np.float32).reshape(-1, 1),
    }
    in_maps = []
    for c in range(NCORES):
        m = dict(shared)
        m["xt"] = np.ascontiguousarray(xt[:, c * LOC:(c + 1) * LOC])
        in_maps.append(m)
    return t, theta, in_maps


def _execute(inputs, **run_kwargs):
    t, theta, in_maps = _prep_inputs(inputs)
    nc = _get_nc(t, theta)
    try:
        res = bass_utils.run_bass_kernel_spmd(
            nc, in_maps, core_ids=list(range(NCORES)), **run_kwargs)
    except ModuleNotFoundError:
        # NTFF trace hook unavailable in this container — run untraced
        os.environ["BASS_NEVER_TRACE"] = "1"
        run_kwargs.pop("trace", None)
        res = bass_utils.run_bass_kernel_spmd(
            nc, in_maps, core_ids=list(range(NCORES)), **run_kwargs)
    out = np.concatenate(
        [res.results[c]["outT"].T for c in range(NCORES)], axis=0)
    return np.ascontiguousarray(out.astype(np.float32)), res


def kernel(**inputs) -> np.ndarray:
    out, _ = _execute(inputs)
    return out


# revision 67
# speedup vs baseline: 1.0790x; 1.0114x over previous
"""Trainium2 Bass kernel for nn_DGraph_GAT (dense latent-graph GraphConv).

Strategy (8 NeuronCores, row-sharded over the 8192 nodes, 1024 nodes/core):
  - feature-major ("transposed") layouts everywhere: tensors are [feat, node]
  - per-core MLP encoder on the local 1024 nodes (layer1 float32r, 2-3 fp16)
  - AllGather of fp16 h and sq(h) (66KB/rank); X1 = h@gw1 and X2 = h1@gw2
    lhsT tiles are recomputed per core from the gathered fp16 data
  - cdist+sigmoid adjacency: an augmented K=34 fp16 matmul produces
    t^2*d2[j,i] directly in PSUM; ACT Sqrt (with +eps bias, clamps fp
    rounding negatives) evacuates PSUM -> fp16 A-buffer; one batched ACT
    Sigmoid pass turns it into A = sigmoid(t*(d+theta)) in place.
    A^T (column block, symmetric) stays fp16-resident in SBUF (16.8MB).
  - GraphConv layers: A^T block streamed through the PE as the moving
    operand against the small gathered X1/X2 matrices (PSUM-accumulated).
  - second tiny AllGather for X2 = h1@gw2, then the output MLP.
  - host side: shard/transpose inputs, gather per-core [16,1024] outputs.
"""

import os

import numpy as np

import concourse.bacc as bacc
import concourse.bass_utils as bass_utils
import concourse.mybir as mybir
import concourse.tile as tile
from concourse.tile_rust import add_dep_helper

F32 = mybir.dt.float32
F32R = mybir.dt.float32r
F16 = mybir.dt.float16
AF = mybir.ActivationFunctionType

NCORES = 8
N = 8192
LOC = N // NCORES          # 1024 nodes per core
JT = N // 128              # 64 j-tiles of 128 global nodes
IN_F, H_F, OUT_F = 512, 256, 32
NCLS = 16
INV = 1.0 / N
EPS_REL = 0.3              # sqrt clamp bias, as a fraction of t^2

# gather-1 flat layout (fp16 elements per rank)
G1_H = OUT_F * LOC         # 32768  h rows
G1_SQ = LOC                # 1024   sq row
G1_TOT = G1_H + G1_SQ      # 33792
G2_TOT = LOC * 16          # 16384  h1 rows (f n)


def _build(t: float, theta: float, sim: bool = False):
    tsq = t * t
    sgn = 1.0 if t >= 0 else -1.0
    nc = bacc.Bacc("TRN2", target_bir_lowering=False, debug=False,
                   enable_asserts=False,
                   num_devices=1 if sim else NCORES)

    def allgather(g_in, g_out, nelem):
        if sim:
            # cost-model build: stand in for the collective with local copies
            # into every rank segment (so unpack deps behave like the real AG)
            for r in range(NCORES):
                nc.sync.dma_start(
                    g_out[r * nelem:(r + 1) * nelem].rearrange(
                        "(o n) -> o n", o=1),
                    g_in[:].rearrange("(o n) -> o n", o=1))
        else:
            nc.gpsimd.collective_compute(
                "AllGather", mybir.AluOpType.bypass,
                replica_groups=[list(range(NCORES))],
                ins=[g_in.opt()], outs=[g_out.opt()])

    # ---- kernel I/O -----------------------------------------------------
    xt_d = nc.dram_tensor("xt", [IN_F, LOC], F32R, kind="ExternalInput")
    w1_d = nc.dram_tensor("w1r", [IN_F, H_F], F32R, kind="ExternalInput")
    b1_d = nc.dram_tensor("b1", [H_F, 1], F32, kind="ExternalInput")
    w2_d = nc.dram_tensor("w2h", [H_F, H_F], F16, kind="ExternalInput")
    b2_d = nc.dram_tensor("b2", [H_F, 1], F32, kind="ExternalInput")
    w3_d = nc.dram_tensor("w3h", [H_F, OUT_F], F16, kind="ExternalInput")
    b3_d = nc.dram_tensor("b3", [OUT_F, 1], F32, kind="ExternalInput")
    gw1_d = nc.dram_tensor("gw1h", [32, 16], F16, kind="ExternalInput")
    gb1_d = nc.dram_tensor("gb1", [16, 1], F32, kind="ExternalInput")
    gw2_d = nc.dram_tensor("gw2h", [16, 8], F16, kind="ExternalInput")
    gb2_d = nc.dram_tensor("gb2", [8, 1], F32, kind="ExternalInput")
    lw1_d = nc.dram_tensor("lw1h", [8, 16], F16, kind="ExternalInput")
    lb1_d = nc.dram_tensor("lb1", [16, 1], F32, kind="ExternalInput")
    lw2_d = nc.dram_tensor("lw2h", [16, 16], F16, kind="ExternalInput")
    lb2_d = nc.dram_tensor("lb2", [NCLS, 1], F32, kind="ExternalInput")
    out_d = nc.dram_tensor("outT", [NCLS, LOC], F32, kind="ExternalOutput")

    with tile.TileContext(nc) as tc:
        with (
            tc.tile_pool(name="dram", bufs=1, space="DRAM") as dram,
            tc.tile_pool(name="outer", bufs=1) as outer,
        ):
            # ---- persistent SBUF tensors -------------------------------
            X_big = outer.tile([128, JT * LOC], F16)      # A^T block / d
            hGT = outer.tile([34, N], F16)                # G lhsT (h,sq,1)
            rhs_G = outer.tile([34, LOC], F16)            # G moving operand
            X1_sb = outer.tile([128, JT * 16], F16)       # X1 lhsT tiles
            X2_sb = outer.tile([128, JT * 8], F16)        # X2 lhsT tiles
            hT3 = outer.tile([OUT_F, LOC], F32)           # local h (fp32)
            hb = outer.tile([OUT_F, LOC], F16)            # local h (fp16)
            sq16 = outer.tile([1, LOC], F16)
            h1T = outer.tile([16, LOC], F16)
            h2T = outer.tile([8, LOC], F16)
            h3T = outer.tile([16, LOC], F16)
            outT = outer.tile([NCLS, LOC], F32)
            gw1_sb = outer.tile([32, 16], F16)
            gb1_sb = outer.tile([16, 1], F32)
            gw2_sb = outer.tile([16, 8], F16)
            gb2_sb = outer.tile([8, 1], F32)
            lw1_sb = outer.tile([8, 16], F16)
            lb1_sb = outer.tile([16, 1], F32)
            lw2_sb = outer.tile([16, 16], F16)
            lb2_sb = outer.tile([NCLS, 1], F32)
            eps_sb = outer.tile([128, 1], F32)
            sigb_sb = outer.tile([128, 1], F32)
            ones32 = outer.tile([32, 1], F16)
            ones_row = outer.tile([1, LOC], F16)
            tsq_row = outer.tile([1, LOC], F16)

            warm_sb = outer.tile([1, 1], F32)
            nc.gpsimd.memset(warm_sb[:], 1.0)
            # preload the sqrt table set during the (ACT-idle) MLP phase;
            # Relu/Identity are filler funcs present in every set
            nc.scalar.activation(warm_sb[:], warm_sb[:], AF.Sqrt)
            nc.gpsimd.memset(eps_sb[:], tsq * EPS_REL)
            nc.gpsimd.memset(sigb_sb[:], t * theta)
            nc.gpsimd.memset(ones32[:], 1.0)
            nc.gpsimd.memset(ones_row[:], 1.0)
            nc.gpsimd.memset(tsq_row[:], tsq)

            g1i = dram.tile([G1_TOT], F16)
            g1o = dram.tile([NCORES * G1_TOT], F16)
            g2ai = dram.tile([G2_TOT // 2], F16)
            g2ao = dram.tile([NCORES * G2_TOT // 2], F16)
            g2bi = dram.tile([G2_TOT // 2], F16)
            g2bo = dram.tile([NCORES * G2_TOT // 2], F16)

            # ================= phase A: local MLP =======================
            with (
                tc.tile_pool(name="mlp", bufs=1) as mlp,
                tc.tile_pool(name="xts", bufs=3) as xts,
                tc.tile_pool(name="mlp_ps", bufs=4, space="PSUM") as mps,
            ):
                w1_sb = mlp.tile([128, 4 * H_F], F32R)
                w2_sb = mlp.tile([128, 2 * H_F], F16)
                w3_sb = mlp.tile([128, 2 * OUT_F], F16)
                b1_sb = mlp.tile([128, 2], F32)
                b2_sb = mlp.tile([128, 2], F32)
                b3_sb = mlp.tile([OUT_F, 1], F32)
                hT1a = mlp.tile([128, LOC], F16)
                hT1b = mlp.tile([128, LOC], F16)
                hT2a = mlp.tile([128, LOC], F16)
                hT2b = mlp.tile([128, LOC], F16)
                sqh = mlp.tile([OUT_F, LOC], F16)



                hT1 = (hT1a, hT1b)
                hT2 = (hT2a, hT2b)
                # layer 1: [512,256] @ xT, float32r; k-outer, 4 accumulators
                pm1 = [[mps.tile([128, 512], F32, tag="pm", name=f"pm1_{m}{n}")
                        for n in range(2)] for m in range(2)]
                for k in range(4):
                    nc.sync.dma_start(w1_sb[:, k * H_F:(k + 1) * H_F],
                                      w1_d[k * 128:(k + 1) * 128, :])
                    xt_k = xts.tile([128, LOC], F32R, tag="xtk")
                    nc.sync.dma_start(xt_k[:], xt_d[k * 128:(k + 1) * 128, :])
                    if k == 0:
                        for kk in range(2):
                            nc.sync.dma_start(b1_sb[:, kk:kk + 1],
                                              b1_d[kk * 128:(kk + 1) * 128, :])
                    if k == 1:
                        for kk in range(2):
                            nc.sync.dma_start(
                                w2_sb[:, kk * H_F:(kk + 1) * H_F],
                                w2_d[kk * 128:(kk + 1) * 128, :])
                            nc.sync.dma_start(b2_sb[:, kk:kk + 1],
                                              b2_d[kk * 128:(kk + 1) * 128, :])
                    if k == 2:
                        for kk in range(2):
                            nc.sync.dma_start(
                                w3_sb[:, kk * OUT_F:(kk + 1) * OUT_F],
                                w3_d[kk * 128:(kk + 1) * 128, :])
                        nc.sync.dma_start(b3_sb[:], b3_d[:])
                    for m in range(2):
                        for n in range(2):
                            nc.tensor.matmul(
                                pm1[m][n][:],
                                w1_sb[:, k * H_F + m * 128:k * H_F + (m + 1) * 128],
                                xt_k[:, n * 512:(n + 1) * 512],
                                start=(k == 0), stop=(k == 3))
                for m in range(2):
                    for n in range(2):
                        nc.scalar.activation(
                            hT1[m][:, n * 512:(n + 1) * 512], pm1[m][n][:],
                            AF.Relu, bias=b1_sb[:, m:m + 1], scale=1.0)
                # layer 2: fp16
                for m in range(2):
                    for n in range(2):
                        pm = mps.tile([128, 512], F32, tag="pm")
                        for k in range(2):
                            nc.tensor.matmul(
                                pm[:],
                                w2_sb[:, k * H_F + m * 128:k * H_F + (m + 1) * 128],
                                hT1[k][:, n * 512:(n + 1) * 512],
                                start=(k == 0), stop=(k == 1))
                        nc.scalar.activation(
                            hT2[m][:, n * 512:(n + 1) * 512], pm[:], AF.Relu,
                            bias=b2_sb[:, m:m + 1], scale=1.0)
                # layer 3: fp16 -> hT3 [32, 1024] fp32 (no relu)
                for n in range(2):
                    pm = mps.tile([OUT_F, 512], F32, tag="pm")
                    for k in range(2):
                        nc.tensor.matmul(
                            pm[:],
                            w3_sb[:, k * OUT_F:(k + 1) * OUT_F],
                            hT2[k][:, n * 512:(n + 1) * 512],
                            start=(k == 0), stop=(k == 1))
                    nc.scalar.activation(
                        hT3[:, n * 512:(n + 1) * 512], pm[:], AF.Identity,
                        bias=b3_sb[:], scale=1.0)

                # fp16 copy of h + sq = sum_f h16^2 (fp16 matmul w/ ones);
                # per node-half so the chain starts on hT3's first half;
                # sq16 and the rhs_G row are written straight from PSUM
                for n in range(2):
                    sl = slice(n * 512, (n + 1) * 512)
                    nc.vector.tensor_copy(hb[:, sl], hT3[:, sl])
                    nc.vector.tensor_mul(sqh[:, sl], hb[:, sl], hb[:, sl])
                    pm = mps.tile([1, 512], F32, tag="pm")
                    nc.tensor.matmul(pm[:], ones32[:], sqh[:, sl],
                                     start=True, stop=True)
                    nc.vector.tensor_copy(sq16[:, sl], pm[:])
                    nc.vector.tensor_scalar_mul(rhs_G[32:33, sl], pm[:], tsq)

            # ================= gather 1 =================================
            nc.sync.dma_start(
                g1i[0:G1_H].rearrange("(f n) -> f n", f=OUT_F), hb[:])
            nc.sync.dma_start(
                g1i[G1_H:G1_H + G1_SQ].rearrange("(o n) -> o n", o=1), sq16[:])

            # moving operand rows: 0-31 = -2 t^2 h_loc (pairs h_glob),
            # 32 = t^2 sq_loc (pairs ones), 33 = t^2 (pairs sq_glob)
            # => psum = t^2 * d2[j, i]   (local data only — build pre-gather)
            nc.vector.tensor_scalar_mul(rhs_G[0:32, :], hT3[:], -2.0 * tsq)

            # static rows + tail weights: fill the collective-latency window
            nc.sync.dma_start(rhs_G[33:34, :], tsq_row[:])
            for r in range(NCORES):
                nc.sync.dma_start(hGT[32:33, r * LOC:(r + 1) * LOC],
                                  ones_row[:])
            nc.gpsimd.dma_start(gw1_sb[:], gw1_d[:])
            nc.gpsimd.dma_start(gb1_sb[:], gb1_d[:])
            nc.gpsimd.dma_start(gw2_sb[:], gw2_d[:])
            nc.gpsimd.dma_start(gb2_sb[:], gb2_d[:])
            nc.gpsimd.dma_start(lw1_sb[:], lw1_d[:])
            nc.gpsimd.dma_start(lb1_sb[:], lb1_d[:])
            nc.gpsimd.dma_start(lw2_sb[:], lw2_d[:])
            nc.gpsimd.dma_start(lb2_sb[:], lb2_d[:])

            allgather(g1i, g1o, G1_TOT)
            g1o_v = g1o[:].rearrange("(r q) -> r q", r=NCORES)
            for r in range(NCORES):
                nc.sync.dma_start(
                    hGT[0:OUT_F, r * LOC:(r + 1) * LOC],
                    g1o_v[r, 0:G1_H].rearrange("(f n) -> f n", f=OUT_F))
                nc.sync.dma_start(
                    hGT[33:34, r * LOC:(r + 1) * LOC],
                    g1o_v[r, G1_H:G1_H + G1_SQ].rearrange(
                        "(o n) -> o n", o=1))

            # ================= phase B: adjacency =======================
            sqrt_insts = []
            with tc.tile_pool(name="g_ps", bufs=2, space="PSUM") as gps:
                # two j-tiles per PSUM tile (4 banks x 2 bufs = all of PSUM);
                # first two groups single-tile so the sqrt sweep starts early
                groups = [(0, 1), (1, 1)] + [(2 * jj, 2)
                                             for jj in range(1, JT // 2)]
                for gi, (lo, nt) in enumerate(groups):
                    pg = gps.tile([128, nt * LOC], F32, tag="pg",
                                  name=f"pg_{gi}")
                    for jt in range(nt):
                        for half in range(2):
                            nc.tensor.matmul(
                                pg[:, jt * LOC + half * 512:
                                   jt * LOC + half * 512 + 512],
                                hGT[:, (lo + jt) * 128:(lo + jt) * 128 + 128],
                                rhs_G[:, half * 512:half * 512 + 512],
                                start=True, stop=True, skip_group_check=True)
                    inst = nc.scalar.activation(
                        X_big[:, lo * LOC:(lo + nt) * LOC], pg[:],
                        AF.Sqrt, bias=eps_sb[:], scale=1.0)
                    sqrt_insts.append(inst)

            with (
                tc.tile_pool(name="acc_ps", bufs=2, space="PSUM") as aps,
                tc.tile_pool(name="x2_ps", bufs=2, space="PSUM") as xps,
                tc.tile_pool(name="x1_ps", bufs=2, space="PSUM") as x1ps,
                tc.tile_pool(name="o2_ps", bufs=2, space="PSUM") as o2ps,
                tc.tile_pool(name="xts2", bufs=4) as xts2,
            ):
                # X1 lhsT tiles from gathered fp16 h — PE is idle during the
                # sigmoid window, so schedule these after the G matmuls
                for g in range(8):
                    px1 = x1ps.tile([128, 128], F32, tag="px1", name=f"px1_{g}")
                    for l in range(8):
                        j = g * 8 + l
                        nc.tensor.matmul(px1[:, l * 16:(l + 1) * 16],
                                         hGT[0:32, j * 128:(j + 1) * 128],
                                         gw1_sb[:], start=True, stop=True,
                                         skip_group_check=True)
                    nc.vector.tensor_copy(
                        X1_sb[:, g * 128:(g + 1) * 128], px1[:])

                # sigmoid + GraphConv1 + gather2, pipelined by node-half:
                # sweep half h of every A-tile, finish o1[h], relu, and fire
                # that half's h1 AllGather while the other half still runs.
                o1a = aps.tile([16, 512], F32, tag="acc")
                o1b = aps.tile([16, 512], F32, tag="acc")
                o1 = (o1a, o1b)
                o2a = o2ps.tile([8, 512], F32, tag="o2")
                o2b = o2ps.tile([8, 512], F32, tag="o2")
                o2 = (o2a, o2b)
                g2io = ((g2ai, g2ao), (g2bi, g2bo))

                def spmm2_tiles(tiles, start_j=None, stop_j=None):
                    insts = []
                    for j in tiles:
                        for ih in range(2):
                            insts.append(nc.tensor.matmul(
                                o2[ih][:], X2_sb[:, j * 8:(j + 1) * 8],
                                X_big[:, j * LOC + ih * 512:
                                      j * LOC + ih * 512 + 512],
                                start=(j == start_j), stop=(j == stop_j)))
                    return insts

                # group-a j-tiles (first 4 of each rank block), chunk-ordered
                ga_tiles = [8 * g + l for g in range(8)
                            for l in range(8) if (8 * g + l) % 8 < 4]

                inter_idx = [0]

                def sig_spmm1_gather(h):
                    last_i2 = [None]
                    # sub-chunks: (tile_lo, n_tiles); final group of sweep-b
                    # split in half so o1's stop-matmul lands ~2us earlier
                    chunks = [(8 * g, 8) for g in range(7)]
                    chunks += [(56, 4), (60, 4)]
                    for g, (lo, nt) in enumerate(chunks):
                        ap4 = X_big[:, lo * LOC:(lo + nt) * LOC].rearrange(
                            "p (l s n) -> p l s n", l=nt, s=2)[
                            :, :, h:h + 1, :]
                        inst = nc.scalar.activation(ap4, ap4, AF.Sigmoid,
                                                    bias=sigb_sb[:], scale=sgn)
                        add_dep_helper(inst.ins, sqrt_insts[-1].ins,
                                       sync=False,
                                       reason="batch ACT table sets")
                        prev = None
                        for j in range(lo, lo + nt):
                            prev = nc.tensor.matmul(
                                o1[h][:], X1_sb[:, j * 16:(j + 1) * 16],
                                X_big[:, j * LOC + h * 512:
                                      j * LOC + h * 512 + 512],
                                start=(j == 0), stop=(j == JT - 1))
                            if last_i2[0] is not None:
                                add_dep_helper(prev.ins, last_i2[0].ins,
                                               sync=False,
                                               reason="pin interleave order")
                                last_i2[0] = None
                        if h == 1 and g >= 4 and nt == 8:
                            # fill PE idle slots of the sigmoid-b sweep with
                            # group-a SpMM2 tiles (X2a landed ~3 chunks ago);
                            # pin the alternation (scheduler would sink these)
                            take = ga_tiles[inter_idx[0]:inter_idx[0] + 4]
                            inter_idx[0] += len(take)
                            if take:
                                ii = spmm2_tiles(take, start_j=ga_tiles[0])
                                add_dep_helper(ii[0].ins, prev.ins,
                                               sync=False,
                                               reason="pin spmm2a interleave")
                                last_i2[0] = ii[-1]
                    nc.scalar.activation(
                        h1T[:, h * 512:(h + 1) * 512], o1[h][:], AF.Relu,
                        bias=gb1_sb[:], scale=INV)
                    nc.sync.dma_start(
                        g2io[h][0][:].rearrange("(f n) -> f n", f=16),
                        h1T[:, h * 512:(h + 1) * 512])
                    allgather(g2io[h][0], g2io[h][1], G2_TOT // 2)

                def x2_prep(h):
                    for r in range(NCORES):
                        h1r = xts2.tile([16, 512], F16, tag="h1r",
                                        name=f"h1r_{h}_{r}")
                        nc.sync.dma_start(
                            h1r[:],
                            g2io[h][1][r * (G2_TOT // 2):
                                       (r + 1) * (G2_TOT // 2)].rearrange(
                                "(f n) -> f n", f=16))
                        px = xps.tile([128, 32], F32, tag="px2",
                                      name=f"px2_{h}_{r}")
                        for ll in range(4):
                            nc.tensor.matmul(px[:, ll * 8:(ll + 1) * 8],
                                             h1r[:, ll * 128:(ll + 1) * 128],
                                             gw2_sb[:], start=True, stop=True,
                                             skip_group_check=True)
                        nc.vector.tensor_copy(
                            X2_sb[:, r * 64 + h * 32:r * 64 + h * 32 + 32],
                            px[:])

                sig_spmm1_gather(0)
                x2_prep(0)              # half-a X2 ready during half-b sweep
                sig_spmm1_gather(1)     # interleaves 20 of 32 group-a tiles
                spmm2_tiles(ga_tiles[inter_idx[0]:])  # rest of group a
                # half-b: unpack + X2 tiles + SpMM2 per rank, pipelined
                gb_last = NCORES * 8 - 1 - 0    # j = 63 is in group-b
                for r in range(NCORES):
                    h1r = xts2.tile([16, 512], F16, tag="h1r",
                                    name=f"h1rb_{r}")
                    nc.sync.dma_start(
                        h1r[:],
                        g2bo[r * (G2_TOT // 2):
                             (r + 1) * (G2_TOT // 2)].rearrange(
                            "(f n) -> f n", f=16))
                    px = xps.tile([128, 32], F32, tag="px2", name=f"px2b_{r}")
                    for ll in range(4):
                        nc.tensor.matmul(px[:, ll * 8:(ll + 1) * 8],
                                         h1r[:, ll * 128:(ll + 1) * 128],
                                         gw2_sb[:], start=True, stop=True,
                                         skip_group_check=True)
                    nc.vector.tensor_copy(
                        X2_sb[:, r * 64 + 32:r * 64 + 64], px[:])
                    spmm2_tiles([r * 8 + 4 + ll for ll in range(4)],
                                stop_j=gb_last)
                for ih in range(2):
                    nc.scalar.activation(
                        h2T[:, ih * 512:(ih + 1) * 512], o2[ih][:], AF.Relu,
                        bias=gb2_sb[:], scale=INV)

                # ---- output MLP ----------------------------------------
                for n in range(2):
                    ph = aps.tile([16, 512], F32, tag="acc")
                    nc.tensor.matmul(ph[:], lw1_sb[:],
                                     h2T[:, n * 512:(n + 1) * 512],
                                     start=True, stop=True)
                    nc.scalar.activation(
                        h3T[:, n * 512:(n + 1) * 512], ph[:], AF.Relu,
                        bias=lb1_sb[:], scale=1.0)
                for n in range(2):
                    po = aps.tile([NCLS, 512], F32, tag="acc")
                    nc.tensor.matmul(po[:], lw2_sb[:],
                                     h3T[:, n * 512:(n + 1) * 512],
                                     start=True, stop=True)
                    nc.scalar.activation(
                        outT[:, n * 512:(n + 1) * 512], po[:], AF.Identity,
                        bias=lb2_sb[:], scale=1.0)
                nc.sync.dma_start(out_d[:], outT[:])

    nc.compile()
    return nc


_CACHE = {}


def _get_nc(t: float, theta: float):
    key = (t, theta)
    if key not in _CACHE:
        _CACHE[key] = _build(t, theta)
    return _CACHE[key]


def _prep_inputs(inputs):
    I = {k: np.asarray(v) for k, v in inputs.items()}
    t = float(I["t"][0, 0])
    theta = float(I["theta"][0, 0])
    xt = np.ascontiguousarray(I["x"].astype(np.float32).T)   # [512, 8192]
    shared = {
        "w1r": np.ascontiguousarray(I["w1"].astype(np.float32)),
        "b1": I["b1"].astype(np.float32).reshape(-1, 1),
        "w2h": np.ascontiguousarray(I["w2"].astype(np.float16)),
        "b2": I["b2"].astype(np.float32).reshape(-1, 1),
        "w3h": np.ascontiguousarray(I["w3"].astype(np.float16)),
        "b3": I["b3"].astype(np.float32).reshape(-1, 1),
        "gw1h": np.ascontiguousarray(I["gw1"].astype(np.float16)),
        "gb1": I["gb1"].astype(np.float32).reshape(-1, 1),
        "gw2h": np.ascontiguousarray(I["gw2"].astype(np.float16)),
        "gb2": I["gb2"].astype(np.float32).reshape(-1, 1),
        "lw1h": np.ascontiguousarray(I["lw1"].astype(np.float16)),
        "lb1": I["lb1"].astype(np.float32).reshape(-1, 1),
        "lw2h": np.ascontiguousarray(I["lw2"].astype(np.float16)),
        "lb2": I["lb2"].astype(np.float32).reshape(-1, 1),
    }
    in_maps = []
    for c in range(NCORES):
        m = dict(shared)
        m["xt"] = np.ascontiguousarray(xt[:, c * LOC:(c + 1) * LOC])
        in_maps.append(m)
    return t, theta, in_maps


def _execute(inputs, **run_kwargs):
    t, theta, in_maps = _prep_inputs(inputs)
    nc = _get_nc(t, theta)
    try:
        res = bass_utils.run_bass_kernel_spmd(
            nc, in_maps, core_ids=list(range(NCORES)), **run_kwargs)
    except ModuleNotFoundError:
        # NTFF trace hook unavailable in this container — run untraced
        os.environ["BASS_NEVER_TRACE"] = "1"
        run_kwargs.pop("trace", None)
        res = bass_utils.run_bass_kernel_spmd(
            nc, in_maps, core_ids=list(range(NCORES)), **run_kwargs)
    out = np.concatenate(
        [res.results[c]["outT"].T for c in range(NCORES)], axis=0)
    return np.ascontiguousarray(out.astype(np.float32)), res


def kernel(**inputs) -> np.ndarray:
    out, _ = _execute(inputs)
    return out


# revision 68
# speedup vs baseline: 1.0793x; 1.0003x over previous
"""Trainium2 Bass kernel for nn_DGraph_GAT (dense latent-graph GraphConv).

Strategy (8 NeuronCores, row-sharded over the 8192 nodes, 1024 nodes/core):
  - feature-major ("transposed") layouts everywhere: tensors are [feat, node]
  - per-core MLP encoder on the local 1024 nodes (layer1 float32r, 2-3 fp16)
  - AllGather of fp16 h and sq(h) (66KB/rank); X1 = h@gw1 and X2 = h1@gw2
    lhsT tiles are recomputed per core from the gathered fp16 data
  - cdist+sigmoid adjacency: an augmented K=34 fp16 matmul produces
    t^2*d2[j,i] directly in PSUM; ACT Sqrt (with +eps bias, clamps fp
    rounding negatives) evacuates PSUM -> fp16 A-buffer; one batched ACT
    Sigmoid pass turns it into A = sigmoid(t*(d+theta)) in place.
    A^T (column block, symmetric) stays fp16-resident in SBUF (16.8MB).
  - GraphConv layers: A^T block streamed through the PE as the moving
    operand against the small gathered X1/X2 matrices (PSUM-accumulated).
  - second tiny AllGather for X2 = h1@gw2, then the output MLP.
  - host side: shard/transpose inputs, gather per-core [16,1024] outputs.
"""

import os

import numpy as np

import concourse.bacc as bacc
import concourse.bass_utils as bass_utils
import concourse.mybir as mybir
import concourse.tile as tile
from concourse.tile_rust import add_dep_helper

F32 = mybir.dt.float32
F32R = mybir.dt.float32r
F16 = mybir.dt.float16
AF = mybir.ActivationFunctionType

NCORES = 8
N = 8192
LOC = N // NCORES          # 1024 nodes per core
JT = N // 128              # 64 j-tiles of 128 global nodes
IN_F, H_F, OUT_F = 512, 256, 32
NCLS = 16
INV = 1.0 / N
EPS_REL = 0.3              # sqrt clamp bias, as a fraction of t^2

# gather-1 flat layout (fp16 elements per rank)
G1_H = OUT_F * LOC         # 32768  h rows
G1_SQ = LOC                # 1024   sq row
G1_TOT = G1_H + G1_SQ      # 33792
G2_TOT = LOC * 16          # 16384  h1 rows (f n)


def _build(t: float, theta: float, sim: bool = False):
    tsq = t * t
    sgn = 1.0 if t >= 0 else -1.0
    nc = bacc.Bacc("TRN2", target_bir_lowering=False, debug=False,
                   enable_asserts=False,
                   num_devices=1 if sim else NCORES)

    def allgather(g_in, g_out, nelem):
        if sim:
            # cost-model build: stand in for the collective with local copies
            # into every rank segment (so unpack deps behave like the real AG)
            for r in range(NCORES):
                nc.sync.dma_start(
                    g_out[r * nelem:(r + 1) * nelem].rearrange(
                        "(o n) -> o n", o=1),
                    g_in[:].rearrange("(o n) -> o n", o=1))
        else:
            nc.gpsimd.collective_compute(
                "AllGather", mybir.AluOpType.bypass,
                replica_groups=[list(range(NCORES))],
                ins=[g_in.opt()], outs=[g_out.opt()])

    # ---- kernel I/O -----------------------------------------------------
    xt_d = nc.dram_tensor("xt", [IN_F, LOC], F32R, kind="ExternalInput")
    w1_d = nc.dram_tensor("w1r", [IN_F, H_F], F32R, kind="ExternalInput")
    b1_d = nc.dram_tensor("b1", [H_F, 1], F32, kind="ExternalInput")
    w2_d = nc.dram_tensor("w2h", [H_F, H_F], F16, kind="ExternalInput")
    b2_d = nc.dram_tensor("b2", [H_F, 1], F32, kind="ExternalInput")
    w3_d = nc.dram_tensor("w3h", [H_F, OUT_F], F16, kind="ExternalInput")
    b3_d = nc.dram_tensor("b3", [OUT_F, 1], F32, kind="ExternalInput")
    gw1_d = nc.dram_tensor("gw1h", [32, 16], F16, kind="ExternalInput")
    gb1_d = nc.dram_tensor("gb1", [16, 1], F32, kind="ExternalInput")
    gw2_d = nc.dram_tensor("gw2h", [16, 8], F16, kind="ExternalInput")
    gb2_d = nc.dram_tensor("gb2", [8, 1], F32, kind="ExternalInput")
    lw1_d = nc.dram_tensor("lw1h", [8, 16], F16, kind="ExternalInput")
    lb1_d = nc.dram_tensor("lb1", [16, 1], F32, kind="ExternalInput")
    lw2_d = nc.dram_tensor("lw2h", [16, 16], F16, kind="ExternalInput")
    lb2_d = nc.dram_tensor("lb2", [NCLS, 1], F32, kind="ExternalInput")
    out_d = nc.dram_tensor("outT", [NCLS, LOC], F32, kind="ExternalOutput")

    with tile.TileContext(nc) as tc:
        with (
            tc.tile_pool(name="dram", bufs=1, space="DRAM") as dram,
            tc.tile_pool(name="outer", bufs=1) as outer,
        ):
            # ---- persistent SBUF tensors -------------------------------
            X_big = outer.tile([128, JT * LOC], F16)      # A^T block / d
            hGT = outer.tile([34, N], F16)                # G lhsT (h,sq,1)
            rhs_G = outer.tile([34, LOC], F16)            # G moving operand
            X1_sb = outer.tile([128, JT * 16], F16)       # X1 lhsT tiles
            X2_sb = outer.tile([128, JT * 8], F16)        # X2 lhsT tiles
            hT3 = outer.tile([OUT_F, LOC], F32)           # local h (fp32)
            hb = outer.tile([OUT_F, LOC], F16)            # local h (fp16)
            sq16 = outer.tile([1, LOC], F16)
            h1T = outer.tile([16, LOC], F16)
            h2T = outer.tile([8, LOC], F16)
            h3T = outer.tile([16, LOC], F16)
            outT = outer.tile([NCLS, LOC], F32)
            gw1_sb = outer.tile([32, 16], F16)
            gb1_sb = outer.tile([16, 1], F32)
            gw2_sb = outer.tile([16, 8], F16)
            gb2_sb = outer.tile([8, 1], F32)
            lw1_sb = outer.tile([8, 16], F16)
            lb1_sb = outer.tile([16, 1], F32)
            lw2_sb = outer.tile([16, 16], F16)
            lb2_sb = outer.tile([NCLS, 1], F32)
            eps_sb = outer.tile([128, 1], F32)
            sigb_sb = outer.tile([128, 1], F32)
            ones32 = outer.tile([32, 1], F16)
            ones_row = outer.tile([1, LOC], F16)
            tsq_row = outer.tile([1, LOC], F16)

            warm_sb = outer.tile([1, 1], F32)
            nc.gpsimd.memset(warm_sb[:], 1.0)
            # preload the sqrt table set during the (ACT-idle) MLP phase;
            # Relu/Identity are filler funcs present in every set
            nc.scalar.activation(warm_sb[:], warm_sb[:], AF.Sqrt)
            nc.gpsimd.memset(eps_sb[:], tsq * EPS_REL)
            nc.gpsimd.memset(sigb_sb[:], t * theta)
            nc.gpsimd.memset(ones32[:], 1.0)
            nc.gpsimd.memset(ones_row[:], 1.0)
            nc.gpsimd.memset(tsq_row[:], tsq)

            g1i = dram.tile([G1_TOT], F16)
            g1o = dram.tile([NCORES * G1_TOT], F16)
            g2ai = dram.tile([G2_TOT // 2], F16)
            g2ao = dram.tile([NCORES * G2_TOT // 2], F16)
            g2bi = dram.tile([G2_TOT // 2], F16)
            g2bo = dram.tile([NCORES * G2_TOT // 2], F16)

            # ================= phase A: local MLP =======================
            with (
                tc.tile_pool(name="mlp", bufs=1) as mlp,
                tc.tile_pool(name="xts", bufs=4) as xts,
                tc.tile_pool(name="mlp_ps", bufs=4, space="PSUM") as mps,
            ):
                w1_sb = mlp.tile([128, 4 * H_F], F32R)
                w2_sb = mlp.tile([128, 2 * H_F], F16)
                w3_sb = mlp.tile([128, 2 * OUT_F], F16)
                b1_sb = mlp.tile([128, 2], F32)
                b2_sb = mlp.tile([128, 2], F32)
                b3_sb = mlp.tile([OUT_F, 1], F32)
                hT1a = mlp.tile([128, LOC], F16)
                hT1b = mlp.tile([128, LOC], F16)
                hT2a = mlp.tile([128, LOC], F16)
                hT2b = mlp.tile([128, LOC], F16)
                sqh = mlp.tile([OUT_F, LOC], F16)



                hT1 = (hT1a, hT1b)
                hT2 = (hT2a, hT2b)
                # layer 1: [512,256] @ xT, float32r; k-outer, 4 accumulators
                pm1 = [[mps.tile([128, 512], F32, tag="pm", name=f"pm1_{m}{n}")
                        for n in range(2)] for m in range(2)]
                for k in range(4):
                    nc.sync.dma_start(w1_sb[:, k * H_F:(k + 1) * H_F],
                                      w1_d[k * 128:(k + 1) * 128, :])
                    xt_k = xts.tile([128, LOC], F32R, tag="xtk")
                    nc.sync.dma_start(xt_k[:], xt_d[k * 128:(k + 1) * 128, :])
                    if k == 0:
                        for kk in range(2):
                            nc.sync.dma_start(b1_sb[:, kk:kk + 1],
                                              b1_d[kk * 128:(kk + 1) * 128, :])
                    if k == 1:
                        for kk in range(2):
                            nc.sync.dma_start(
                                w2_sb[:, kk * H_F:(kk + 1) * H_F],
                                w2_d[kk * 128:(kk + 1) * 128, :])
                            nc.sync.dma_start(b2_sb[:, kk:kk + 1],
                                              b2_d[kk * 128:(kk + 1) * 128, :])
                    if k == 2:
                        for kk in range(2):
                            nc.sync.dma_start(
                                w3_sb[:, kk * OUT_F:(kk + 1) * OUT_F],
                                w3_d[kk * 128:(kk + 1) * 128, :])
                        nc.sync.dma_start(b3_sb[:], b3_d[:])
                    for m in range(2):
                        for n in range(2):
                            nc.tensor.matmul(
                                pm1[m][n][:],
                                w1_sb[:, k * H_F + m * 128:k * H_F + (m + 1) * 128],
                                xt_k[:, n * 512:(n + 1) * 512],
                                start=(k == 0), stop=(k == 3))
                for m in range(2):
                    for n in range(2):
                        nc.scalar.activation(
                            hT1[m][:, n * 512:(n + 1) * 512], pm1[m][n][:],
                            AF.Relu, bias=b1_sb[:, m:m + 1], scale=1.0)
                # layer 2: fp16
                for m in range(2):
                    for n in range(2):
                        pm = mps.tile([128, 512], F32, tag="pm")
                        for k in range(2):
                            nc.tensor.matmul(
                                pm[:],
                                w2_sb[:, k * H_F + m * 128:k * H_F + (m + 1) * 128],
                                hT1[k][:, n * 512:(n + 1) * 512],
                                start=(k == 0), stop=(k == 1))
                        nc.scalar.activation(
                            hT2[m][:, n * 512:(n + 1) * 512], pm[:], AF.Relu,
                            bias=b2_sb[:, m:m + 1], scale=1.0)
                # layer 3: fp16 -> hT3 [32, 1024] fp32 (no relu)
                for n in range(2):
                    pm = mps.tile([OUT_F, 512], F32, tag="pm")
                    for k in range(2):
                        nc.tensor.matmul(
                            pm[:],
                            w3_sb[:, k * OUT_F:(k + 1) * OUT_F],
                            hT2[k][:, n * 512:(n + 1) * 512],
                            start=(k == 0), stop=(k == 1))
                    nc.scalar.activation(
                        hT3[:, n * 512:(n + 1) * 512], pm[:], AF.Identity,
                        bias=b3_sb[:], scale=1.0)

                # fp16 copy of h + sq = sum_f h16^2 (fp16 matmul w/ ones);
                # per node-half so the chain starts on hT3's first half;
                # sq16 and the rhs_G row are written straight from PSUM
                for n in range(2):
                    sl = slice(n * 512, (n + 1) * 512)
                    nc.vector.tensor_copy(hb[:, sl], hT3[:, sl])
                    nc.vector.tensor_mul(sqh[:, sl], hb[:, sl], hb[:, sl])
                    pm = mps.tile([1, 512], F32, tag="pm")
                    nc.tensor.matmul(pm[:], ones32[:], sqh[:, sl],
                                     start=True, stop=True)
                    nc.vector.tensor_copy(sq16[:, sl], pm[:])
                    nc.vector.tensor_scalar_mul(rhs_G[32:33, sl], pm[:], tsq)

            # ================= gather 1 =================================
            nc.sync.dma_start(
                g1i[0:G1_H].rearrange("(f n) -> f n", f=OUT_F), hb[:])
            nc.sync.dma_start(
                g1i[G1_H:G1_H + G1_SQ].rearrange("(o n) -> o n", o=1), sq16[:])

            # moving operand rows: 0-31 = -2 t^2 h_loc (pairs h_glob),
            # 32 = t^2 sq_loc (pairs ones), 33 = t^2 (pairs sq_glob)
            # => psum = t^2 * d2[j, i]   (local data only — build pre-gather)
            nc.vector.tensor_scalar_mul(rhs_G[0:32, :], hT3[:], -2.0 * tsq)

            # static rows + tail weights: fill the collective-latency window
            nc.sync.dma_start(rhs_G[33:34, :], tsq_row[:])
            for r in range(NCORES):
                nc.sync.dma_start(hGT[32:33, r * LOC:(r + 1) * LOC],
                                  ones_row[:])
            nc.gpsimd.dma_start(gw1_sb[:], gw1_d[:])
            nc.gpsimd.dma_start(gb1_sb[:], gb1_d[:])
            nc.gpsimd.dma_start(gw2_sb[:], gw2_d[:])
            nc.gpsimd.dma_start(gb2_sb[:], gb2_d[:])
            nc.gpsimd.dma_start(lw1_sb[:], lw1_d[:])
            nc.gpsimd.dma_start(lb1_sb[:], lb1_d[:])
            nc.gpsimd.dma_start(lw2_sb[:], lw2_d[:])
            nc.gpsimd.dma_start(lb2_sb[:], lb2_d[:])

            allgather(g1i, g1o, G1_TOT)
            g1o_v = g1o[:].rearrange("(r q) -> r q", r=NCORES)
            for r in range(NCORES):
                nc.sync.dma_start(
                    hGT[0:OUT_F, r * LOC:(r + 1) * LOC],
                    g1o_v[r, 0:G1_H].rearrange("(f n) -> f n", f=OUT_F))
                nc.sync.dma_start(
                    hGT[33:34, r * LOC:(r + 1) * LOC],
                    g1o_v[r, G1_H:G1_H + G1_SQ].rearrange(
                        "(o n) -> o n", o=1))

            # ================= phase B: adjacency =======================
            sqrt_insts = []
            with tc.tile_pool(name="g_ps", bufs=2, space="PSUM") as gps:
                # two j-tiles per PSUM tile (4 banks x 2 bufs = all of PSUM);
                # first two groups single-tile so the sqrt sweep starts early
                groups = [(0, 1), (1, 1)] + [(2 * jj, 2)
                                             for jj in range(1, JT // 2)]
                for gi, (lo, nt) in enumerate(groups):
                    pg = gps.tile([128, nt * LOC], F32, tag="pg",
                                  name=f"pg_{gi}")
                    for jt in range(nt):
                        for half in range(2):
                            nc.tensor.matmul(
                                pg[:, jt * LOC + half * 512:
                                   jt * LOC + half * 512 + 512],
                                hGT[:, (lo + jt) * 128:(lo + jt) * 128 + 128],
                                rhs_G[:, half * 512:half * 512 + 512],
                                start=True, stop=True, skip_group_check=True)
                    inst = nc.scalar.activation(
                        X_big[:, lo * LOC:(lo + nt) * LOC], pg[:],
                        AF.Sqrt, bias=eps_sb[:], scale=1.0)
                    sqrt_insts.append(inst)

            with (
                tc.tile_pool(name="acc_ps", bufs=2, space="PSUM") as aps,
                tc.tile_pool(name="x2_ps", bufs=2, space="PSUM") as xps,
                tc.tile_pool(name="x1_ps", bufs=2, space="PSUM") as x1ps,
                tc.tile_pool(name="o2_ps", bufs=2, space="PSUM") as o2ps,
                tc.tile_pool(name="xts2", bufs=4) as xts2,
            ):
                # X1 lhsT tiles from gathered fp16 h — PE is idle during the
                # sigmoid window, so schedule these after the G matmuls
                for g in range(8):
                    px1 = x1ps.tile([128, 128], F32, tag="px1", name=f"px1_{g}")
                    for l in range(8):
                        j = g * 8 + l
                        nc.tensor.matmul(px1[:, l * 16:(l + 1) * 16],
                                         hGT[0:32, j * 128:(j + 1) * 128],
                                         gw1_sb[:], start=True, stop=True,
                                         skip_group_check=True)
                    nc.vector.tensor_copy(
                        X1_sb[:, g * 128:(g + 1) * 128], px1[:])

                # sigmoid + GraphConv1 + gather2, pipelined by node-half:
                # sweep half h of every A-tile, finish o1[h], relu, and fire
                # that half's h1 AllGather while the other half still runs.
                o1a = aps.tile([16, 512], F32, tag="acc")
                o1b = aps.tile([16, 512], F32, tag="acc")
                o1 = (o1a, o1b)
                o2a = o2ps.tile([8, 512], F32, tag="o2")
                o2b = o2ps.tile([8, 512], F32, tag="o2")
                o2 = (o2a, o2b)
                g2io = ((g2ai, g2ao), (g2bi, g2bo))

                def spmm2_tiles(tiles, start_j=None, stop_j=None):
                    insts = []
                    for j in tiles:
                        for ih in range(2):
                            insts.append(nc.tensor.matmul(
                                o2[ih][:], X2_sb[:, j * 8:(j + 1) * 8],
                                X_big[:, j * LOC + ih * 512:
                                      j * LOC + ih * 512 + 512],
                                start=(j == start_j), stop=(j == stop_j)))
                    return insts

                # group-a j-tiles (first 4 of each rank block), chunk-ordered
                ga_tiles = [8 * g + l for g in range(8)
                            for l in range(8) if (8 * g + l) % 8 < 4]

                inter_idx = [0]

                def sig_spmm1_gather(h):
                    last_i2 = [None]
                    # sub-chunks: (tile_lo, n_tiles); final group of sweep-b
                    # split in half so o1's stop-matmul lands ~2us earlier
                    chunks = [(8 * g, 8) for g in range(7)]
                    chunks += [(56, 4), (60, 4)]
                    for g, (lo, nt) in enumerate(chunks):
                        ap4 = X_big[:, lo * LOC:(lo + nt) * LOC].rearrange(
                            "p (l s n) -> p l s n", l=nt, s=2)[
                            :, :, h:h + 1, :]
                        inst = nc.scalar.activation(ap4, ap4, AF.Sigmoid,
                                                    bias=sigb_sb[:], scale=sgn)
                        add_dep_helper(inst.ins, sqrt_insts[-1].ins,
                                       sync=False,
                                       reason="batch ACT table sets")
                        prev = None
                        for j in range(lo, lo + nt):
                            prev = nc.tensor.matmul(
                                o1[h][:], X1_sb[:, j * 16:(j + 1) * 16],
                                X_big[:, j * LOC + h * 512:
                                      j * LOC + h * 512 + 512],
                                start=(j == 0), stop=(j == JT - 1))
                            if last_i2[0] is not None:
                                add_dep_helper(prev.ins, last_i2[0].ins,
                                               sync=False,
                                               reason="pin interleave order")
                                last_i2[0] = None
                        if h == 1 and g >= 4 and nt == 8:
                            # fill PE idle slots of the sigmoid-b sweep with
                            # group-a SpMM2 tiles (X2a landed ~3 chunks ago);
                            # pin the alternation (scheduler would sink these)
                            take = ga_tiles[inter_idx[0]:inter_idx[0] + 4]
                            inter_idx[0] += len(take)
                            if take:
                                ii = spmm2_tiles(take, start_j=ga_tiles[0])
                                add_dep_helper(ii[0].ins, prev.ins,
                                               sync=False,
                                               reason="pin spmm2a interleave")
                                last_i2[0] = ii[-1]
                    nc.scalar.activation(
                        h1T[:, h * 512:(h + 1) * 512], o1[h][:], AF.Relu,
                        bias=gb1_sb[:], scale=INV)
                    nc.sync.dma_start(
                        g2io[h][0][:].rearrange("(f n) -> f n", f=16),
                        h1T[:, h * 512:(h + 1) * 512])
                    allgather(g2io[h][0], g2io[h][1], G2_TOT // 2)

                def x2_prep(h):
                    for r in range(NCORES):
                        h1r = xts2.tile([16, 512], F16, tag="h1r",
                                        name=f"h1r_{h}_{r}")
                        nc.sync.dma_start(
                            h1r[:],
                            g2io[h][1][r * (G2_TOT // 2):
                                       (r + 1) * (G2_TOT // 2)].rearrange(
                                "(f n) -> f n", f=16))
                        px = xps.tile([128, 32], F32, tag="px2",
                                      name=f"px2_{h}_{r}")
                        for ll in range(4):
                            nc.tensor.matmul(px[:, ll * 8:(ll + 1) * 8],
                                             h1r[:, ll * 128:(ll + 1) * 128],
                                             gw2_sb[:], start=True, stop=True,
                                             skip_group_check=True)
                        nc.vector.tensor_copy(
                            X2_sb[:, r * 64 + h * 32:r * 64 + h * 32 + 32],
                            px[:])

                sig_spmm1_gather(0)
                x2_prep(0)              # half-a X2 ready during half-b sweep
                sig_spmm1_gather(1)     # interleaves 20 of 32 group-a tiles
                spmm2_tiles(ga_tiles[inter_idx[0]:])  # rest of group a
                # half-b: unpack + X2 tiles + SpMM2 per rank, pipelined
                gb_last = NCORES * 8 - 1 - 0    # j = 63 is in group-b
                for r in range(NCORES):
                    h1r = xts2.tile([16, 512], F16, tag="h1r",
                                    name=f"h1rb_{r}")
                    nc.sync.dma_start(
                        h1r[:],
                        g2bo[r * (G2_TOT // 2):
                             (r + 1) * (G2_TOT // 2)].rearrange(
                            "(f n) -> f n", f=16))
                    px = xps.tile([128, 32], F32, tag="px2", name=f"px2b_{r}")
                    for ll in range(4):
                        nc.tensor.matmul(px[:, ll * 8:(ll + 1) * 8],
                                         h1r[:, ll * 128:(ll + 1) * 128],
                                         gw2_sb[:], start=True, stop=True,
                                         skip_group_check=True)
                    nc.vector.tensor_copy(
                        X2_sb[:, r * 64 + 32:r * 64 + 64], px[:])
                    spmm2_tiles([r * 8 + 4 + ll for ll in range(4)],
                                stop_j=gb_last)
                for ih in range(2):
                    nc.scalar.activation(
                        h2T[:, ih * 512:(ih + 1) * 512], o2[ih][:], AF.Relu,
                        bias=gb2_sb[:], scale=INV)

                # ---- output MLP ----------------------------------------
                for n in range(2):
                    ph = aps.tile([16, 512], F32, tag="acc")
                    nc.tensor.matmul(ph[:], lw1_sb[:],
                                     h2T[:, n * 512:(n + 1) * 512],
                                     start=True, stop=True)
                    nc.scalar.activation(
                        h3T[:, n * 512:(n + 1) * 512], ph[:], AF.Relu,
                        bias=lb1_sb[:], scale=1.0)
                for n in range(2):
                    po = aps.tile([NCLS, 512], F32, tag="acc")
                    nc.tensor.matmul(po[:], lw2_sb[:],
                                     h3T[:, n * 512:(n + 1) * 512],
                                     start=True, stop=True)
                    nc.scalar.activation(
                        outT[:, n * 512:(n + 1) * 512], po[:], AF.Identity,
                        bias=lb2_sb[:], scale=1.0)
                nc.sync.dma_start(out_d[:], outT[:])

    nc.compile()
    return nc


_CACHE = {}


def _get_nc(t: float, theta: float):
    key = (t, theta)
    if key not in _CACHE:
        _CACHE[key] = _build(t, theta)
    return _CACHE[key]


def _prep_inputs(inputs):
    I = {k: np.asarray(v) for k, v in inputs.items()}
    t = float(I["t"][0, 0])
    theta = float(I["theta"][0, 0])
    xt = np.ascontiguousarray(I["x"].astype(np.float32).T)   # [512, 8192]
    shared = {
        "w1r": np.ascontiguousarray(I["w1"].astype(np.float32)),
        "b1": I["b1"].astype(np.float32).reshape(-1, 1),
        "w2h": np.ascontiguousarray(I["w2"].astype(np.float16)),
        "b2": I["b2"].astype(np.float32).reshape(-1, 1),
        "w3h": np.ascontiguousarray(I["w3"].astype(np.float16)),
        "b3": I["b3"].astype(np.float32).reshape(-1, 1),
        "gw1h": np.ascontiguousarray(I["gw1"].astype(np.float16)),
        "gb1": I["gb1"].astype(np.float32).reshape(-1, 1),
        "gw2h": np.ascontiguousarray(I["gw2"].astype(np.float16)),
        "gb2": I["gb2"].astype(np.float32).reshape(-1, 1),
        "lw1h": np.ascontiguousarray(I["lw1"].astype(np.float16)),
        "lb1": I["lb1"].astype(np.float32).reshape(-1, 1),
        "lw2h": np.ascontiguousarray(I["lw2"].astype(np.float16)),
        "lb2": I["lb2"].astype(np.float32).reshape(-1, 1),
    }
    in_maps = []
    for c in range(NCORES):
        m = dict(shared)
        m["xt"] = np.ascontiguousarray(xt[:, c * LOC:(c + 1) * LOC])
        in_maps.append(m)
    return t, theta, in_maps


def _execute(inputs, **run_kwargs):
    t, theta, in_maps = _prep_inputs(inputs)
    nc = _get_nc(t, theta)
    try:
        res = bass_utils.run_bass_kernel_spmd(
            nc, in_maps, core_ids=list(range(NCORES)), **run_kwargs)
    except ModuleNotFoundError:
        # NTFF trace hook unavailable in this container — run untraced
        os.environ["BASS_NEVER_TRACE"] = "1"
        run_kwargs.pop("trace", None)
        res = bass_utils.run_bass_kernel_spmd(
            nc, in_maps, core_ids=list(range(NCORES)), **run_kwargs)
    out = np.concatenate(
        [res.results[c]["outT"].T for c in range(NCORES)], axis=0)
    return np.ascontiguousarray(out.astype(np.float32)), res


def kernel(**inputs) -> np.ndarray:
    out, _ = _execute(inputs)
    return out
